# revision 1
# baseline (speedup 1.0000x reference)
"""GAT (2-layer, 4-head then 1-head) on 8 Trainium2 NeuronCores.

Strategy
--------
- Nodes are permuted: globally degree-sorted, dealt round-robin to 8 cores
  (edge balance + nearly-identical degree profiles per core), then each
  core's nodes are laid out in 128-node dst tiles. Tiles are degree-uniform,
  so per-dst edge lists pad to the tile max with tiny waste.
- Edges land in a "slot grid" [128 dst x K slots] per tile: slot-chunk c is
  128 edges whose partition IS the dst row. The aggregation matmul then has
  an identity stationary operand (no per-chunk one-hot masks at all).
- Per-edge messages are fetched with dma_gather (int16 indices). The node
  table is split at the core-5/6 row boundary so both halves fit in int16
  (rows 0..30720 via table A view, rows 30721.. via offset view). Sentinel
  rows (0 and last) have attention logits of -1e30 so padded slots get
  weight exp(-inf) = 0 and contribute nothing, including to the softmax
  denominator.
- Softmax is computed without the max-subtraction (values are O(1); the
  normalization cancels exactly): w_e = exp(leakyrelu(x)) = max(exp(x),
  exp(0.2 x)), accumulated per dst in PSUM along with the denominator, and
  divided once per node.
- Layer outputs are transformed (W2 / W_out) per tile; the layer-2 node
  table is AllGather'd across cores between layers (as is the layer-1
  table after the sharded x @ W1 phase).
"""

import numpy as np

import concourse.bacc as bacc
import concourse.mybir as mybir
import concourse.tile as tile
from concourse.bass_utils import run_bass_kernel_spmd

F32 = mybir.dt.float32
BF16 = mybir.dt.float16  # NB: fp16 (renamed var kept)
I16 = mybir.dt.int16

IN_CH = 128
HID = 32
HEADS = 4
OUT_CH = 112
NEG_SLOPE = 0.2
NEG_BIG = -60000.0

# Layer-1 table row: [h(128) | al_src(4) | al_dst(4) | one(1) | pad] = 256 bf16 (512B)
T1_COLS = 256
T1_USED = 141  # 132 (4x[h|1]) + 8 al
# Layer-2 table row: [z(32) | as2(1) | ad2(1) | one(1) | pad] = 128 bf16 (256B)
T2_COLS = 128
T2_USED = 35

N_CORES = 8
GCAP = 56


def _prep(x, edge_index, W1, a_src1, a_dst1, b1, W2, a_src2, a_dst2, b2, W_out, b_out):
    """Host-side graph preprocessing. Returns (meta, per-core inputs)."""
    N = x.shape[0]
    E = edge_index.shape[1]
    per_core = -(-N // (N_CORES * 128)) * 128
    n_pad = per_core * N_CORES
    NT = per_core // 128
    nrows = n_pad + 2  # + 2 sentinel rows
    b_base = 1 + 6 * per_core  # first table row owned by core 6
    assert b_base - 1 <= 32767 and nrows - b_base <= 32767

    src = np.concatenate([edge_index[0], np.arange(N, dtype=np.int64)])
    dst = np.concatenate([edge_index[1], np.arange(N, dtype=np.int64)])
    E2 = src.shape[0]

    deg = np.bincount(dst, minlength=n_pad)
    order = np.argsort(deg, kind="stable")  # ascending degree, pads first
    # deal round-robin: global rank i -> core i%8, position i//8
    rank = np.empty(n_pad, np.int64)
    rank[order] = np.arange(n_pad)
    core_of = rank % N_CORES
    pos_in_core = rank // N_CORES
    grow = core_of * per_core + pos_in_core      # global row-1 (0-based over n_pad)
    trow = 1 + grow                               # table row of each node
    # perm_rows[g] = node sitting at global row g
    perm_rows = np.empty(n_pad, np.int64)
    perm_rows[grow] = np.arange(n_pad)

    sr = trow[src]
    dr = grow[dst]
    grp = (sr >= b_base).astype(np.int64)  # 0 = A half, 1 = B half

    # sort edges by (dst row, group); compute slot index within each run
    eorder = np.lexsort((grp, dr))
    dr_s = dr[eorder]
    sr_s = sr[eorder]
    g_s = grp[eorder]
    key = dr_s * 2 + g_s
    newrun = np.empty(E2, bool)
    newrun[0] = True
    newrun[1:] = key[1:] != key[:-1]
    run_id = np.cumsum(newrun) - 1
    run_start = np.flatnonzero(newrun)
    slot = np.arange(E2) - run_start[run_id]

    a_cnt = np.bincount(dr_s[g_s == 0], minlength=n_pad)
    b_cnt = np.bincount(dr_s[g_s == 1], minlength=n_pad)
    # per-(core,tile) maxima, unified across cores
    Ka = a_cnt.reshape(N_CORES, NT, 128).max(axis=(0, 2))
    Kb = b_cnt.reshape(N_CORES, NT, 128).max(axis=(0, 2))
    Kt = Ka + Kb

    # adaptive groups: consecutive tiles, sum of slots <= GCAP
    groups = []
    t = 0
    while t < NT:
        e = t
        tot = 0
        while e < NT and (e == t or tot + Ka[e] + Kb[e] <= GCAP):
            tot += Ka[e] + Kb[e]
            e += 1
        groups.append((t, e))
        t = e
    base_a = np.zeros(NT, np.int64)
    base_b = np.zeros(NT, np.int64)
    ga_base = []
    gb_base = []
    off = 0
    for (t0, t1) in groups:
        ga_base.append(off)
        for t in range(t0, t1):
            base_a[t] = off
            off += 128 * Ka[t]
        gb_base.append(off)
        for t in range(t0, t1):
            base_b[t] = off
            off += 128 * Kb[t]
    totidx = off
    sent_b_local = nrows - 1 - b_base

    # default stream = sentinels
    default = np.zeros(totidx, np.int16)
    for t in range(NT):
        default[base_a[t]:base_a[t] + 128 * Ka[t]] = 0
        default[base_b[t]:base_b[t] + 128 * Kb[t]] = sent_b_local
    streams = np.tile(default, (N_CORES, 1))

    e_core = dr_s // per_core
    loc = dr_s % per_core
    tl = loc // 128
    p = loc % 128
    posA = base_a[tl] + slot * 128 + p
    posB = base_b[tl] + slot * 128 + p
    pos = np.where(g_s == 0, posA, posB)
    val = np.where(g_s == 0, sr_s, sr_s - b_base).astype(np.int16)
    streams[e_core, pos] = val

    # wrap for dma_gather: wrapped[p, j] = flat[j*16 + p%16]
    assert totidx % 16 == 0
    idx_wrapped = np.empty((N_CORES, 128, totidx // 16), np.int16)
    for c in range(N_CORES):
        w16 = streams[c].reshape(-1, 16).T  # [16, totidx/16]
        idx_wrapped[c] = np.tile(w16, (8, 1))

    # x slices (table-row order per core)
    xp = np.zeros((n_pad, IN_CH), np.float32)
    xp[:N] = np.asarray(x, np.float32)
    x_slices = np.empty((N_CORES, IN_CH, per_core), np.float32)
    for c in range(N_CORES):
        nodes = perm_rows[c * per_core:(c + 1) * per_core]
        x_slices[c] = xp[nodes].T

    # weight packs
    W1 = np.asarray(W1, np.float32)
    Bsrc = np.zeros((HEADS * HID, HEADS), np.float32)
    Bdst = np.zeros((HEADS * HID, HEADS), np.float32)
    for h in range(HEADS):
        Bsrc[h * HID:(h + 1) * HID, h] = np.asarray(a_src1[h], np.float32)
        Bdst[h * HID:(h + 1) * HID, h] = np.asarray(a_dst1[h], np.float32)
    W1cols = []
    for h in range(HEADS):
        W1cols.append(W1[:, h * HID:(h + 1) * HID])
        W1cols.append(np.zeros((IN_CH, 1), np.float32))  # ones-slot
    W1big = np.concatenate(W1cols + [W1 @ Bsrc, W1 @ Bdst], axis=1)  # [128, 140]
    W2 = np.asarray(W2, np.float32)
    W2big = np.concatenate(
        [W2, W2 @ np.asarray(a_src2, np.float32).T, W2 @ np.asarray(a_dst2, np.float32).T],
        axis=1,
    )  # [128, 34]
    b1v = np.asarray(b1, np.float32).reshape(HEADS, HID)
    b1i = np.zeros((HEADS, HID + 1), np.float32)
    b1i[:, :HID] = b1v
    b1_rep = np.tile(b1i.reshape(1, -1), (128, 1))                            # [128,132]
    b2_rep = np.zeros((128, HID + 2), np.float32)
    b2_rep[:, :HID] = np.asarray(b2, np.float32)[None, :]
    bout_rep = np.tile(np.asarray(b_out, np.float32)[None, :], (128, 1))     # [128,112]
    ident = np.eye(128, dtype=np.float32)

    bf16 = np.float16
    sent1 = np.zeros((1, 144), bf16)
    sent1[0, 132:140] = NEG_BIG
    sent2 = np.zeros((1, 36), bf16)
    sent2[0, 32:34] = NEG_BIG

    meta = dict(
        N=N, E2=E2, n_pad=n_pad, per_core=per_core, NT=NT, nrows=nrows,
        b_base=b_base, Ka=Ka.tolist(), Kb=Kb.tolist(),
        base_a=base_a.tolist(), base_b=base_b.tolist(), totidx=totidx,
        ga_base=ga_base, gb_base=gb_base, groups=groups,
        perm_rows=perm_rows,
    )
    shared = dict(
        W1big=W1big, W2big=W2big.astype(bf16), Wout=np.asarray(W_out, np.float32).astype(bf16),
        b1_rep=b1_rep, b2_rep=b2_rep, bout_rep=bout_rep, ident=ident.astype(bf16),
        sent1=sent1, sent2=sent2,
    )
    in_maps = []
    for c in range(N_CORES):
        m = dict(shared)
        m["x_slice"] = np.ascontiguousarray(x_slices[c])
        m["idx_flat"] = np.ascontiguousarray(idx_wrapped[c])
        in_maps.append(m)
    return meta, in_maps


def _build(meta):
    per_core, NT, nrows, b_base = meta["per_core"], meta["NT"], meta["nrows"], meta["b_base"]
    Ka, Kb = meta["Ka"], meta["Kb"]
    base_a, base_b, totidx = meta["base_a"], meta["base_b"], meta["totidx"]
    ga_base, gb_base, groups = meta["ga_base"], meta["gb_base"], meta["groups"]

    nc = bacc.Bacc("TRN2", num_devices=N_CORES, num_swdge_queues=4,
                   dynamic_dma_scratch_size=65536)

    x_slice = nc.dram_tensor("x_slice", [IN_CH, per_core], F32, kind="ExternalInput")
    idx_flat = nc.dram_tensor("idx_flat", [128, totidx // 16], I16, kind="ExternalInput")
    W1big_d = nc.dram_tensor("W1big", [128, 140], F32, kind="ExternalInput")
    W2big_d = nc.dram_tensor("W2big", [128, HID + 2], BF16, kind="ExternalInput")
    Wout_d = nc.dram_tensor("Wout", [HID, OUT_CH], BF16, kind="ExternalInput")
    b1_d = nc.dram_tensor("b1_rep", [128, 132], F32, kind="ExternalInput")
    b2_d = nc.dram_tensor("b2_rep", [128, HID + 2], F32, kind="ExternalInput")
    bout_d = nc.dram_tensor("bout_rep", [128, OUT_CH], F32, kind="ExternalInput")
    ident_d = nc.dram_tensor("ident", [128, 128], BF16, kind="ExternalInput")
    sent1_d = nc.dram_tensor("sent1", [1, 144], BF16, kind="ExternalInput")
    sent2_d = nc.dram_tensor("sent2", [1, 36], BF16, kind="ExternalInput")

    T1_own = nc.dram_tensor("T1_own", [per_core, T1_COLS], BF16, kind="Internal")
    T1d = nc.dram_tensor("T1d", [per_core, 8], BF16, kind="Internal")
    T2d = nc.dram_tensor("T2d", [per_core, 2], BF16, kind="Internal")
    T1_sh = nc.dram_tensor("T1_sh", [nrows, T1_COLS], BF16, kind="Internal", addr_space="Shared")
    T2_own = nc.dram_tensor("T2_own", [per_core, T2_COLS], BF16, kind="Internal")
    T2_sh = nc.dram_tensor("T2_sh", [nrows, T2_COLS], BF16, kind="Internal", addr_space="Shared")
    out_d = nc.dram_tensor("out", [per_core, OUT_CH], F32, kind="ExternalOutput")

    rgroups = [list(range(N_CORES))]
    qctr = [0]

    def qn():
        q = qctr[0] % 4
        qctr[0] += 1
        return q

    with tile.TileContext(nc) as tc:
        with (
            tc.tile_pool(name="const", bufs=1) as cp,
            tc.tile_pool(name="xa", bufs=2) as xap,
            tc.tile_pool(name="stage", bufs=3) as sp,
            tc.tile_pool(name="g1", bufs=2) as g1p,
            tc.tile_pool(name="g2", bufs=2) as g2p,
            tc.tile_pool(name="small", bufs=4) as smp,
            tc.tile_pool(name="rhs", bufs=2) as rp,
            tc.tile_pool(name="epi", bufs=3) as ep,
            tc.tile_pool(name="psa", bufs=3, space="PSUM") as ppa,
            tc.tile_pool(name="psm", bufs=3, space="PSUM") as ppm,
            tc.tile_pool(name="psy", bufs=2, space="PSUM") as ppy,
        ):
            # ---- consts to SBUF
            W1big = cp.tile([128, 140], F32)
            nc.sync.dma_start(out=W1big[:], in_=W1big_d[:])
            W2big = cp.tile([128, HID + 2], BF16)
            nc.sync.dma_start(out=W2big[:], in_=W2big_d[:])
            Wout = cp.tile([HID, OUT_CH], BF16)
            nc.sync.dma_start(out=Wout[:], in_=Wout_d[:])
            b1r = cp.tile([128, 132], F32)
            nc.sync.dma_start(out=b1r[:], in_=b1_d[:])
            b2r = cp.tile([128, HID + 2], F32)
            nc.sync.dma_start(out=b2r[:], in_=b2_d[:])
            boutr = cp.tile([128, OUT_CH], F32)
            nc.sync.dma_start(out=boutr[:], in_=bout_d[:])
            ident = cp.tile([128, 128], BF16)
            nc.sync.dma_start(out=ident[:], in_=ident_d[:])
            idxs = cp.tile([128, totidx // 16], I16)
            nc.sync.dma_start(out=idxs[:], in_=idx_flat[:])

            # ---- phase A: own node tiles -> T1_own
            for t in range(NT):
                xa = xap.tile([128, 128], F32)
                nc.sync.dma_start(out=xa[:], in_=x_slice[:, t * 128:(t + 1) * 128])
                ps = ppa.tile([128, 140], F32, tag="agg")
                nc.tensor.matmul(out=ps[:], lhsT=xa[:], rhs=W1big[:], start=True, stop=True)
                hb = sp.tile([128, T1_COLS], BF16)
                nc.vector.tensor_copy(out=hb[:, 0:140], in_=ps[:])
                hb_v = hb[:, 0:132].rearrange("p (h j) -> p h j", h=HEADS)
                nc.vector.memset(hb_v[:, :, HID:HID + 1], 1.0)
                nc.vector.memset(hb[:, 140:T1_COLS], 0.0)
                nc.sync.dma_start(out=T1_own[t * 128:(t + 1) * 128, :], in_=hb[:])
                hd = sp.tile([128, 8], BF16, tag="hd")
                nc.vector.tensor_copy(out=hd[:], in_=ps[:, 132:140])
                nc.sync.dma_start(out=T1d[t * 128:(t + 1) * 128, :], in_=hd[:])

            # ---- allgather T1 + sentinel pokes
            nc.gpsimd.collective_compute(
                "AllGather", mybir.AluOpType.bypass, replica_groups=rgroups,
                ins=[T1_own[:]], outs=[T1_sh[1:1 + N_CORES * per_core, :]],
            )
            s1 = cp.tile([1, 144], BF16)
            nc.sync.dma_start(out=s1[:], in_=sent1_d[:])
            nc.sync.dma_start(out=T1_sh[0:1, 0:144], in_=s1[:])
            nc.sync.dma_start(out=T1_sh[nrows - 1:nrows, 0:144], in_=s1[:])

            # ---- layer 1 edge phase (grouped super-gathers)
            for gi, (t0, t1) in enumerate(groups):
                tiles = list(range(t0, t1))
                SA = sum(Ka[t] for t in tiles)
                SB = sum(Kb[t] for t in tiles)
                if SA + SB == 0:
                    continue
                GA = g1p.tile([128, max(SA, 1), T1_COLS], BF16, tag="GA")
                if SA:
                    o = ga_base[gi]
                    nc.gpsimd.dma_gather(
                        GA[:, 0:SA, :], T1_sh[:], idxs[:, o // 16: o // 16 + SA * 8],
                        128 * SA, 128 * SA, T1_COLS,
                        queue_num=qn(), single_packet=False)
                GB = g1p.tile([128, max(SB, 1), T1_COLS], BF16, tag="GB")
                if SB:
                    o = gb_base[gi]
                    nc.gpsimd.dma_gather(
                        GB[:, 0:SB, :], T1_sh[b_base:nrows, :], idxs[:, o // 16: o // 16 + SB * 8],
                        128 * SB, 128 * SB, T1_COLS,
                        queue_num=qn(), single_packet=False)
                for t in tiles:
                    ka, kb = Ka[t], Kb[t]
                    kt = ka + kb
                    if kt == 0:
                        continue
                    aoff = (base_a[t] - ga_base[gi]) // 128
                    boff = (base_b[t] - gb_base[gi]) // 128
                    al8 = smp.tile([128, 8], BF16, tag="al8")
                    nc.sync.dma_start(out=al8[:], in_=T1d[t * 128:(t + 1) * 128, :])
                    ps = ppa.tile([128, HEADS * (HID + 1)], F32, tag="agg")
                    rhs = rp.tile([128, kt, HEADS * (HID + 1)], BF16, tag="rhs1")
                    rhs_v = rhs[:].rearrange("p k (h j) -> p k h j", h=HEADS)
                    for (G, goff, nk, ro) in ((GA, aoff, ka, 0), (GB, boff, kb, ka)):
                        if nk == 0:
                            continue
                        Gs = G[:, goff:goff + nk, :]
                        xl = smp.tile([128, nk, HEADS], BF16, tag="xl")
                        nc.vector.tensor_tensor(
                            out=xl[:], in0=Gs[:, :, 132:136],
                            in1=al8[:, None, 4:8].to_broadcast([128, nk, HEADS]),
                            op=mybir.AluOpType.add)
                        e1 = smp.tile([128, nk, HEADS], BF16, tag="e1")
                        nc.scalar.activation(e1[:], xl[:], mybir.ActivationFunctionType.Exp)
                        e2 = smp.tile([128, nk, HEADS], BF16, tag="e2")
                        nc.scalar.activation(e2[:], xl[:], mybir.ActivationFunctionType.Exp, scale=NEG_SLOPE)
                        w = smp.tile([128, nk, HEADS], BF16, tag="w")
                        nc.vector.tensor_tensor(out=w[:], in0=e1[:], in1=e2[:], op=mybir.AluOpType.max)
                        nc.vector.tensor_tensor(
                            out=rhs_v[:, ro:ro + nk, :, :],
                            in0=Gs[:, :, 0:132].rearrange("p k (h j) -> p k h j", h=HEADS),
                            in1=w[:, :, :, None].to_broadcast([128, nk, HEADS, HID + 1]),
                            op=mybir.AluOpType.mult)
                    for cch in range(kt):
                        nc.tensor.matmul(out=ps[:], lhsT=ident[:], rhs=rhs[:, cch, :],
                                         start=(cch == 0), stop=(cch == kt - 1))
                    # epilogue: divide, +b1, ELU
                    ps_v = ps[:].rearrange("p (h j) -> p h j", h=HEADS)
                    rec = smp.tile([128, HEADS], F32, tag="rec")
                    nc.vector.reciprocal(out=rec[:], in_=ps_v[:, :, HID])
                    y1 = ppy.tile([128, 128], F32, tag="y")
                    nc.vector.tensor_tensor(
                        out=y1[:].rearrange("p (h j) -> p h j", h=HEADS),
                        in0=ps_v[:, :, 0:HID],
                        in1=rec[:, :, None].to_broadcast([128, HEADS, HID]),
                        op=mybir.AluOpType.mult)
                    nc.vector.tensor_tensor(
                        out=y1[:].rearrange("p (h j) -> p h j", h=HEADS),
                        in0=y1[:].rearrange("p (h j) -> p h j", h=HEADS),
                        in1=b1r[:].rearrange("p (h j) -> p h j", h=HEADS)[:, :, 0:HID],
                        op=mybir.AluOpType.add)
                    m1 = ep.tile([128, 128], F32, tag="m1")
                    nc.vector.tensor_scalar(out=m1[:], in0=y1[:], scalar1=0.0, scalar2=None,
                                            op0=mybir.AluOpType.min)
                    eE = ep.tile([128, 128], F32, tag="eE")
                    nc.scalar.activation(eE[:], m1[:], mybir.ActivationFunctionType.Exp)
                    r1 = ep.tile([128, 128], F32, tag="r1")
                    nc.vector.tensor_scalar(out=r1[:], in0=y1[:], scalar1=0.0, scalar2=-1.0,
                                            op0=mybir.AluOpType.max, op1=mybir.AluOpType.add)
                    h2 = ep.tile([128, 128], BF16, tag="h2")
                    nc.vector.tensor_tensor(out=h2[:], in0=eE[:], in1=r1[:], op=mybir.AluOpType.add)
                    # transpose h2, z = h2 @ W2big
                    pt = ppm.tile([128, 128], BF16, tag="misc")
                    nc.tensor.transpose(out=pt[:], in_=h2[:], identity=ident[:])
                    h2T = ep.tile([128, 128], BF16, tag="h2T")
                    nc.vector.tensor_copy(out=h2T[:], in_=pt[:])
                    psz = ppm.tile([128, HID + 2], F32, tag="misc")
                    nc.tensor.matmul(out=psz[:], lhsT=h2T[:], rhs=W2big[:], start=True, stop=True)
                    t2b = sp.tile([128, T2_COLS], BF16, tag="t2b")
                    nc.vector.tensor_tensor(out=t2b[:, 0:HID + 2], in0=psz[:], in1=b2r[:],
                                            op=mybir.AluOpType.add)
                    nc.vector.memset(t2b[:, HID + 2:HID + 3], 1.0)
                    nc.vector.memset(t2b[:, HID + 3:T2_COLS], 0.0)
                    nc.sync.dma_start(out=T2_own[t * 128:(t + 1) * 128, :], in_=t2b[:])
                    td2 = sp.tile([128, 2], BF16, tag="td2")
                    nc.vector.tensor_tensor(out=td2[:], in0=psz[:, 32:34], in1=b2r[:, 32:34],
                                            op=mybir.AluOpType.add)
                    nc.sync.dma_start(out=T2d[t * 128:(t + 1) * 128, :], in_=td2[:])

            # ---- allgather T2 + sentinel pokes
            nc.gpsimd.collective_compute(
                "AllGather", mybir.AluOpType.bypass, replica_groups=rgroups,
                ins=[T2_own[:]], outs=[T2_sh[1:1 + N_CORES * per_core, :]],
            )
            s2 = cp.tile([1, 36], BF16)
            nc.sync.dma_start(out=s2[:], in_=sent2_d[:])
            nc.sync.dma_start(out=T2_sh[0:1, 0:36], in_=s2[:])
            nc.sync.dma_start(out=T2_sh[nrows - 1:nrows, 0:36], in_=s2[:])

            # ---- layer 2 edge phase + output (grouped super-gathers)
            for gi, (t0, t1) in enumerate(groups):
                tiles2 = list(range(t0, t1))
                SA = sum(Ka[t] for t in tiles2)
                SB = sum(Kb[t] for t in tiles2)
                if SA + SB == 0:
                    continue
                GA2 = g2p.tile([128, max(SA, 1), T2_COLS], BF16, tag="GA2")
                if SA:
                    o = ga_base[gi]
                    nc.gpsimd.dma_gather(
                        GA2[:, 0:SA, :], T2_sh[:], idxs[:, o // 16: o // 16 + SA * 8],
                        128 * SA, 128 * SA, T2_COLS,
                        queue_num=qn(), single_packet=False)
                GB2 = g2p.tile([128, max(SB, 1), T2_COLS], BF16, tag="GB2")
                if SB:
                    o = gb_base[gi]
                    nc.gpsimd.dma_gather(
                        GB2[:, 0:SB, :], T2_sh[b_base:nrows, :], idxs[:, o // 16: o // 16 + SB * 8],
                        128 * SB, 128 * SB, T2_COLS,
                        queue_num=qn(), single_packet=False)
                for t in tiles2:
                    ka, kb = Ka[t], Kb[t]
                    kt = ka + kb
                    if kt == 0:
                        continue
                    aoff = (base_a[t] - ga_base[gi]) // 128
                    boff = (base_b[t] - gb_base[gi]) // 128
                    ad2 = smp.tile([128, 2], BF16, tag="ad2")
                    nc.sync.dma_start(out=ad2[:], in_=T2d[t * 128:(t + 1) * 128, :])
                    ps2 = ppa.tile([128, T2_USED], F32, tag="agg")
                    rhs2 = rp.tile([128, kt, T2_USED], BF16, tag="rhs2")
                    for (G2, goff, nk, ro) in ((GA2, aoff, ka, 0), (GB2, boff, kb, ka)):
                        if nk == 0:
                            continue
                        Gs = G2[:, goff:goff + nk, :]
                        xl2 = smp.tile([128, nk, 1], BF16, tag="xl2")
                        nc.vector.tensor_tensor(
                            out=xl2[:], in0=Gs[:, :, 32:33],
                            in1=ad2[:, None, 1:2].to_broadcast([128, nk, 1]),
                            op=mybir.AluOpType.add)
                        e1b = smp.tile([128, nk, 1], BF16, tag="e1b")
                        nc.scalar.activation(e1b[:], xl2[:], mybir.ActivationFunctionType.Exp)
                        e2b = smp.tile([128, nk, 1], BF16, tag="e2b")
                        nc.scalar.activation(e2b[:], xl2[:], mybir.ActivationFunctionType.Exp, scale=NEG_SLOPE)
                        w2 = smp.tile([128, nk, 1], BF16, tag="w2")
                        nc.vector.tensor_tensor(out=w2[:], in0=e1b[:], in1=e2b[:], op=mybir.AluOpType.max)
                        nc.vector.tensor_tensor(
                            out=rhs2[:, ro:ro + nk, :],
                            in0=Gs[:, :, 0:T2_USED],
                            in1=w2[:, :, :].to_broadcast([128, nk, T2_USED]),
                            op=mybir.AluOpType.mult)
                    for cch in range(kt):
                        nc.tensor.matmul(out=ps2[:], lhsT=ident[:], rhs=rhs2[:, cch, :],
                                         start=(cch == 0), stop=(cch == kt - 1))
                    rec2 = smp.tile([128, 1], F32, tag="rec2")
                    nc.vector.reciprocal(out=rec2[:], in_=ps2[:, HID + 2:HID + 3])
                    y2 = ppy.tile([128, HID], F32, tag="y")
                    nc.vector.tensor_tensor(
                        out=y2[:], in0=ps2[:, 0:HID],
                        in1=rec2[:].to_broadcast([128, HID]),
                        op=mybir.AluOpType.mult)
                    m2 = ep.tile([128, HID], F32, tag="m2")
                    nc.vector.tensor_scalar(out=m2[:], in0=y2[:], scalar1=0.0, scalar2=None,
                                            op0=mybir.AluOpType.min)
                    eE2 = ep.tile([128, HID], F32, tag="eE2")
                    nc.scalar.activation(eE2[:], m2[:], mybir.ActivationFunctionType.Exp)
                    r2 = ep.tile([128, HID], F32, tag="r2")
                    nc.vector.tensor_scalar(out=r2[:], in0=y2[:], scalar1=0.0, scalar2=-1.0,
                                            op0=mybir.AluOpType.max, op1=mybir.AluOpType.add)
                    h3 = ep.tile([128, HID], BF16, tag="h3")
                    nc.vector.tensor_tensor(out=h3[:], in0=eE2[:], in1=r2[:], op=mybir.AluOpType.add)
                    pt2 = ppm.tile([128, 128], BF16, tag="misc")
                    nc.tensor.transpose(out=pt2[:HID, :], in_=h3[:], identity=ident[:])
                    h3T = ep.tile([HID, 128], BF16, tag="h3T")
                    nc.vector.tensor_copy(out=h3T[:], in_=pt2[:HID, :])
                    psf = ppm.tile([128, OUT_CH], F32, tag="misc")
                    nc.tensor.matmul(out=psf[:], lhsT=h3T[:], rhs=Wout[:], start=True, stop=True)
                    outf = ep.tile([128, OUT_CH], F32, tag="outf")
                    nc.vector.tensor_tensor(out=outf[:], in0=psf[:], in1=boutr[:],
                                            op=mybir.AluOpType.add)
                    nc.sync.dma_start(out=out_d[t * 128:(t + 1) * 128, :], in_=outf[:])

    nc.compile()
    return nc


def _run(inputs, trace=False):
    meta, in_maps = _prep(**inputs)
    nc = _build(meta)
    res = run_bass_kernel_spmd(nc, in_maps, core_ids=list(range(N_CORES)), trace=trace)
    per_core = meta["per_core"]
    outg = np.concatenate([res.results[c]["out"] for c in range(N_CORES)], axis=0)
    # global row g holds node perm_rows[g]
    out_nodes = np.empty((meta["n_pad"], OUT_CH), np.float32)
    out_nodes[meta["perm_rows"]] = outg
    return out_nodes[:meta["N"]], res


def kernel(**inputs):
    out, _ = _run(inputs, trace=False)
    return out



# revision 5
# speedup vs baseline: 1.7522x; 1.7522x over previous
"""GAT (2-layer, 4-head then 1-head) on 8 Trainium2 NeuronCores.

Strategy (v3 — dense one-hot chunks, group-level batching)
----------------------------------------------------------
- Nodes degree-sorted and dealt round-robin to 8 cores; each core's 5120
  nodes form 40 dst tiles of 128.
- Edges (self-loops excluded) are packed DENSELY per (tile, table-half)
  into 128-edge chunks (pad ~9%). Aggregation multiplies each chunk by a
  one-hot lhsT whose column d selects the partitions holding edges of
  dst d, accumulating Sum_e w_e*[h_e | 1] per dst in PSUM.
- Per-edge al_dst: alD[p] = onehotT_c (contract dst) al8_own — computed
  for ALL chunks of a group into one PSUM bank, one matmul per chunk.
  onehotT is streamed from DRAM; the aggregation one-hot is generated on
  device (DVE is_equal(codes, iota)).
- All per-edge DVE work (al add, exp, max, message multiply) happens at
  GROUP granularity (~36 chunks per instruction), not per tile; only the
  aggregation matmuls and the epilogue are per tile.
- Self-loops never gathered: extra rhs slots + identity-matmul chunks
  fed from SBUF-resident h_own / z_own.
- Softmax without max-subtraction (exp(lrelu(x)) = max(exp x, exp .2x));
  denominators ride the same one-hot matmul (w columns). Pad slots have
  all-zero one-hot columns.
- Node tables: L1 rows [h(128)|al_src(4)|pad] 512B; L2 rows
  [z+b2(32)|as2(1)|pad] 256B. Table rows are numbered CHUNK-MAJOR
  (pos<2560 first for all cores, then the rest) so each half-table
  AllGather has a contiguous output and can overlap compute. int16
  gather indices via the A/B table split at row 30721.
"""

import numpy as np

import concourse.bacc as bacc
import concourse.mybir as mybir
import concourse.tile as tile
from concourse.bass_utils import run_bass_kernel_spmd

F32 = mybir.dt.float32
F16 = mybir.dt.float16
I16 = mybir.dt.int16

IN_CH = 128
HID = 32
HEADS = 4
OUT_CH = 112
NEG_SLOPE = 0.2

T1_COLS = 256
T2_COLS = 128

N_CORES = 8
GCAP = 36
PAD_CODE = 200
HROWS = 2560  # rows per core per allgather chunk (2 chunks)


def _prep(x, edge_index, W1, a_src1, a_dst1, b1, W2, a_src2, a_dst2, b2, W_out, b_out):
    N = x.shape[0]
    per_core = -(-N // (N_CORES * 128)) * 128
    n_pad = per_core * N_CORES
    NT = per_core // 128
    nrows = n_pad + 2
    b_base = 1 + 6 * per_core
    assert b_base - 1 <= 32767 and nrows - b_base <= 32767
    assert per_core == 2 * HROWS

    src = np.asarray(edge_index[0], np.int64)
    dst = np.asarray(edge_index[1], np.int64)
    E = src.shape[0]

    deg = np.bincount(dst, minlength=n_pad)
    order = np.argsort(deg, kind="stable")
    rank = np.empty(n_pad, np.int64)
    rank[order] = np.arange(n_pad)
    coreid = rank % N_CORES
    pos = rank // N_CORES
    grow = coreid * per_core + pos
    # chunk-major table rows: all cores' pos<HROWS first, then the rest
    trow = 1 + (pos // HROWS) * (N_CORES * HROWS) + coreid * HROWS + pos % HROWS
    perm_rows = np.empty(n_pad, np.int64)
    perm_rows[grow] = np.arange(n_pad)

    sr = trow[src]
    dr = grow[dst]
    gB = sr >= b_base
    core = dr // per_core
    tl = (dr % per_core) // 128
    lane = dr % 128

    EaT = np.zeros((N_CORES, NT), np.int64)
    EbT = np.zeros((N_CORES, NT), np.int64)
    np.add.at(EaT, (core[~gB], tl[~gB]), 1)
    np.add.at(EbT, (core[gB], tl[gB]), 1)
    chA = (-(-EaT // 128)).max(axis=0)
    chB = (-(-EbT // 128)).max(axis=0)

    groups = []
    t = 0
    while t < NT:
        e = t
        tot = 0
        while e < NT and (e == t or tot + chA[e] + chB[e] <= GCAP):
            tot += chA[e] + chB[e]
            e += 1
        groups.append((t, e))
        t = e

    aoff = np.zeros(NT, np.int64)
    boff = np.zeros(NT, np.int64)
    gc0 = []
    gSA = []
    gSB = []
    C = 0
    g_of_tile = np.zeros(NT, np.int64)
    for gi, (t0, t1) in enumerate(groups):
        sa = int(chA[t0:t1].sum())
        sb = int(chB[t0:t1].sum())
        gc0.append(C)
        gSA.append(sa)
        gSB.append(sb)
        off = 0
        for t in range(t0, t1):
            g_of_tile[t] = gi
            aoff[t] = off
            off += chA[t]
        off = 0
        for t in range(t0, t1):
            boff[t] = off
            off += chB[t]
        C += sa + sb
    totidx = C * 128
    assert totidx % 16 == 0

    SENT_A = 0
    SENT_B = nrows - 1 - b_base
    idx_streams = np.zeros((N_CORES, C, 128), np.int16)
    for gi in range(len(groups)):
        idx_streams[:, gc0[gi]:gc0[gi] + gSA[gi], :] = SENT_A
        idx_streams[:, gc0[gi] + gSA[gi]:gc0[gi] + gSA[gi] + gSB[gi], :] = SENT_B
    codes_streams = np.full((N_CORES, C, 128), PAD_CODE, np.int16)

    cbaseA = np.array([gc0[g_of_tile[t]] + aoff[t] for t in range(NT)])
    cbaseB = np.array([gc0[g_of_tile[t]] + gSA[g_of_tile[t]] + boff[t] for t in range(NT)])

    key = (core * NT + tl) * 2 + gB.astype(np.int64)
    eorder = np.argsort(key, kind="stable")
    ks = key[eorder]
    newrun = np.ones(E, bool)
    newrun[1:] = ks[1:] != ks[:-1]
    run_start = np.flatnonzero(newrun)
    run_id = np.cumsum(newrun) - 1
    j = np.arange(E) - run_start[run_id]
    cs = core[eorder]
    tls = tl[eorder]
    gs = gB[eorder]
    cidx = np.where(gs, cbaseB[tls], cbaseA[tls]) + j // 128
    idx_streams[cs, cidx, j % 128] = np.where(gs, sr[eorder] - b_base, sr[eorder]).astype(np.int16)
    codes_streams[cs, cidx, j % 128] = lane[eorder]

    idx_wrapped = np.empty((N_CORES, 128, totidx // 16), np.int16)
    for c in range(N_CORES):
        w16 = idx_streams[c].reshape(-1, 16).T
        idx_wrapped[c] = np.tile(w16, (8, 1))

    f16 = np.float16
    codes_pc = np.transpose(codes_streams, (0, 2, 1)).astype(f16)
    d_ar = np.arange(128, dtype=np.int16)[:, None, None]
    ohT = np.empty((N_CORES, 128, C, 128), f16)
    for c in range(N_CORES):
        ohT[c] = (codes_streams[c][None, :, :] == d_ar).astype(f16)

    xp = np.zeros((n_pad, IN_CH), np.float32)
    xp[:N] = np.asarray(x, np.float32)
    x_slices = np.empty((N_CORES, IN_CH, per_core), np.float32)
    for c in range(N_CORES):
        x_slices[c] = xp[perm_rows[c * per_core:(c + 1) * per_core]].T

    W1 = np.asarray(W1, np.float32)
    Bsrc = np.zeros((HEADS * HID, HEADS), np.float32)
    Bdst = np.zeros((HEADS * HID, HEADS), np.float32)
    for h in range(HEADS):
        Bsrc[h * HID:(h + 1) * HID, h] = np.asarray(a_src1[h], np.float32)
        Bdst[h * HID:(h + 1) * HID, h] = np.asarray(a_dst1[h], np.float32)
    W1big = np.concatenate([W1, W1 @ Bsrc, W1 @ Bdst], axis=1)
    W2 = np.asarray(W2, np.float32)
    W2big = np.concatenate(
        [W2, W2 @ np.asarray(a_src2, np.float32).T, W2 @ np.asarray(a_dst2, np.float32).T],
        axis=1,
    )
    b1_rep = np.tile(np.asarray(b1, np.float32)[None, :], (128, 1))
    b2_rep = np.zeros((128, HID + 2), np.float32)
    b2_rep[:, :HID] = np.asarray(b2, np.float32)[None, :]
    bout_rep = np.tile(np.asarray(b_out, np.float32)[None, :], (128, 1))
    ident = np.eye(128, dtype=f16)
    iota = np.tile(np.arange(128, dtype=f16)[None, :], (128, 1))

    meta = dict(
        N=N, n_pad=n_pad, per_core=per_core, NT=NT, nrows=nrows, b_base=b_base,
        chA=chA.tolist(), chB=chB.tolist(), groups=groups, gc0=gc0, gSA=gSA,
        gSB=gSB, aoff=aoff.tolist(), boff=boff.tolist(), C=C, totidx=totidx,
        perm_rows=perm_rows,
    )
    shared = dict(
        W1big=W1big, W2big=W2big.astype(f16), Wout=np.asarray(W_out, np.float32).astype(f16),
        b1_rep=b1_rep, b2_rep=b2_rep, bout_rep=bout_rep, ident=ident, iota=iota,
    )
    in_maps = []
    for c in range(N_CORES):
        m = dict(shared)
        m["x_slice"] = np.ascontiguousarray(x_slices[c])
        m["idx_flat"] = np.ascontiguousarray(idx_wrapped[c])
        m["codes"] = np.ascontiguousarray(codes_pc[c])
        m["ohT"] = np.ascontiguousarray(ohT[c])
        in_maps.append(m)
    return meta, in_maps


def _build(meta):
    per_core, NT, nrows, b_base = meta["per_core"], meta["NT"], meta["nrows"], meta["b_base"]
    chA, chB = meta["chA"], meta["chB"]
    groups, gc0, gSA, gSB = meta["groups"], meta["gc0"], meta["gSA"], meta["gSB"]
    aoff, boff, C = meta["aoff"], meta["boff"], meta["C"]
    totidx = meta["totidx"]

    nc = bacc.Bacc("TRN2", num_devices=N_CORES, num_swdge_queues=4,
                   dynamic_dma_scratch_size=32768)

    x_slice = nc.dram_tensor("x_slice", [IN_CH, per_core], F32, kind="ExternalInput")
    idx_flat = nc.dram_tensor("idx_flat", [128, totidx // 16], I16, kind="ExternalInput")
    codes_d = nc.dram_tensor("codes", [128, C], F16, kind="ExternalInput")
    ohT_d = nc.dram_tensor("ohT", [128, C, 128], F16, kind="ExternalInput")
    W1big_d = nc.dram_tensor("W1big", [128, 136], F32, kind="ExternalInput")
    W2big_d = nc.dram_tensor("W2big", [128, HID + 2], F16, kind="ExternalInput")
    Wout_d = nc.dram_tensor("Wout", [HID, OUT_CH], F16, kind="ExternalInput")
    b1_d = nc.dram_tensor("b1_rep", [128, 128], F32, kind="ExternalInput")
    b2_d = nc.dram_tensor("b2_rep", [128, HID + 2], F32, kind="ExternalInput")
    bout_d = nc.dram_tensor("bout_rep", [128, OUT_CH], F32, kind="ExternalInput")
    ident_d = nc.dram_tensor("ident", [128, 128], F16, kind="ExternalInput")
    iota_d = nc.dram_tensor("iota", [128, 128], F16, kind="ExternalInput")

    T1_own = nc.dram_tensor("T1_own", [per_core, T1_COLS], F16, kind="Internal")
    T1_sh = nc.dram_tensor("T1_sh", [nrows, T1_COLS], F16, kind="Internal", addr_space="Shared")
    T2_own = nc.dram_tensor("T2_own", [per_core, T2_COLS], F16, kind="Internal")
    T2_sh = nc.dram_tensor("T2_sh", [nrows, T2_COLS], F16, kind="Internal", addr_space="Shared")
    out_d = nc.dram_tensor("out", [per_core, OUT_CH], F32, kind="ExternalOutput")

    rgroups = [list(range(N_CORES))]
    qctr = [0]

    def qn():
        q = qctr[0] % 4
        qctr[0] += 1
        return q

    def allgather(own, sh, half):
        r0 = half * HROWS
        o0 = 1 + half * N_CORES * HROWS
        nc.gpsimd.collective_compute(
            "AllGather", mybir.AluOpType.bypass, replica_groups=rgroups,
            ins=[own[r0:r0 + HROWS, :]], outs=[sh[o0:o0 + N_CORES * HROWS, :]],
        )

    # per-group slot -> tile map
    slot_tile = []
    for gi, (t0, t1) in enumerate(groups):
        st = []
        for t in range(t0, t1):
            st += [t] * chA[t]
        for t in range(t0, t1):
            st += [t] * chB[t]
        slot_tile.append(st)

    with tile.TileContext(nc) as tc:
        with (
            tc.tile_pool(name="const", bufs=1) as cp,
            tc.tile_pool(name="persist", bufs=1) as pp,
            tc.tile_pool(name="xa", bufs=2) as xap,
            tc.tile_pool(name="stage", bufs=3) as sp,
            tc.tile_pool(name="idxp", bufs=3) as ixp,
            tc.tile_pool(name="gath", bufs=3) as gp,
            tc.tile_pool(name="ot", bufs=2) as otp,
            tc.tile_pool(name="oh", bufs=2) as ohp,
            tc.tile_pool(name="rhs", bufs=2) as rp,
            tc.tile_pool(name="small", bufs=3) as smp,
            tc.tile_pool(name="epi", bufs=2) as ep,
            tc.tile_pool(name="psa", bufs=3, space="PSUM") as ppa,
            tc.tile_pool(name="psal", bufs=2, space="PSUM") as pal,
            tc.tile_pool(name="psm", bufs=3, space="PSUM") as ppm,
        ):
            # ---- consts
            W1big = cp.tile([128, 136], F32)
            nc.sync.dma_start(out=W1big[:], in_=W1big_d[:])
            W2big = cp.tile([128, HID + 2], F16)
            nc.sync.dma_start(out=W2big[:], in_=W2big_d[:])
            Wout = cp.tile([HID, OUT_CH], F16)
            nc.sync.dma_start(out=Wout[:], in_=Wout_d[:])
            b1r = cp.tile([128, 128], F32)
            nc.sync.dma_start(out=b1r[:], in_=b1_d[:])
            b2r = cp.tile([128, HID + 2], F32)
            nc.sync.dma_start(out=b2r[:], in_=b2_d[:])
            boutr = cp.tile([128, OUT_CH], F32)
            nc.sync.dma_start(out=boutr[:], in_=bout_d[:])
            ident = cp.tile([128, 128], F16)
            nc.sync.dma_start(out=ident[:], in_=ident_d[:])
            iota = cp.tile([128, 128], F16)
            nc.sync.dma_start(out=iota[:], in_=iota_d[:])
            codes = cp.tile([128, C], F16)
            nc.sync.dma_start(out=codes[:], in_=codes_d[:])

            h_own = pp.tile([128, NT * 128], F16)
            alT1 = pp.tile([128, NT * 8], F16)
            z_own = pp.tile([128, NT * HID], F16)
            alT2 = pp.tile([128, NT * 2], F16)
            wself1 = pp.tile([128, NT, HEADS], F16)
            wself2 = pp.tile([128, NT, 1], F16)

            zs1 = cp.tile([1, T1_COLS], F16)
            nc.vector.memset(zs1[:], 0.0)
            nc.sync.dma_start(out=T1_sh[0:1, :], in_=zs1[:])
            nc.sync.dma_start(out=T1_sh[nrows - 1:nrows, :], in_=zs1[:])
            nc.sync.dma_start(out=T2_sh[0:1, :], in_=zs1[:, 0:T2_COLS])
            nc.sync.dma_start(out=T2_sh[nrows - 1:nrows, :], in_=zs1[:, 0:T2_COLS])

            # ---- phase A
            for t in range(NT):
                xa = xap.tile([128, 128], F32)
                nc.sync.dma_start(out=xa[:], in_=x_slice[:, t * 128:(t + 1) * 128])
                ps = ppa.tile([128, 136], F32, tag="agg")
                nc.tensor.matmul(out=ps[:], lhsT=xa[:], rhs=W1big[:], start=True, stop=True)
                hb = sp.tile([128, T1_COLS], F16, tag="hb")
                nc.vector.tensor_copy(out=hb[:, 0:132], in_=ps[:, 0:132])
                nc.sync.dma_start(out=T1_own[t * 128:(t + 1) * 128, :], in_=hb[:])
                nc.vector.tensor_copy(out=h_own[:, t * 128:(t + 1) * 128], in_=ps[:, 0:128])
                nc.vector.tensor_copy(out=alT1[:, t * 8:t * 8 + 8], in_=ps[:, 128:136])
                if t == NT // 2 - 1:
                    allgather(T1_own, T1_sh, 0)
            allgather(T1_own, T1_sh, 1)

            alT1v = alT1[:].rearrange("p (t e) -> p t e", t=NT)
            xls = smp.tile([128, NT, HEADS], F16, tag="xls")
            nc.vector.tensor_tensor(out=xls[:], in0=alT1v[:, :, 0:4], in1=alT1v[:, :, 4:8],
                                    op=mybir.AluOpType.add)
            e1s = smp.tile([128, NT, HEADS], F16, tag="e1s")
            nc.scalar.activation(e1s[:], xls[:], mybir.ActivationFunctionType.Exp)
            e2s = smp.tile([128, NT, HEADS], F16, tag="e2s")
            nc.scalar.activation(e2s[:], xls[:], mybir.ActivationFunctionType.Exp, scale=NEG_SLOPE)
            nc.vector.tensor_tensor(out=wself1[:], in0=e1s[:], in1=e2s[:], op=mybir.AluOpType.max)

            # ---- layer 1
            for gi, (t0, t1) in enumerate(groups):
                SA, SB = gSA[gi], gSB[gi]
                S = SA + SB
                nt = t1 - t0
                c0 = gc0[gi]
                idxg = ixp.tile([128, S * 8], I16, tag="idx")
                nc.sync.dma_start(out=idxg[:], in_=idx_flat[:, c0 * 8:(c0 + S) * 8])
                G = gp.tile([128, S, T1_COLS], F16, tag="G1")
                if SA:
                    nc.gpsimd.dma_gather(
                        G[:, 0:SA, :], T1_sh[:], idxg[:, 0:SA * 8],
                        128 * SA, 128 * SA, T1_COLS, queue_num=qn(), single_packet=False)
                if SB:
                    nc.gpsimd.dma_gather(
                        G[:, SA:S, :], T1_sh[b_base:nrows, :], idxg[:, SA * 8:S * 8],
                        128 * SB, 128 * SB, T1_COLS, queue_num=qn(), single_packet=False)
                OT = otp.tile([128, S, 128], F16, tag="OT")
                nc.sync.dma_start(out=OT[:], in_=ohT_d[:, c0:c0 + S, :])
                OH = ohp.tile([128, S, 128], F16, tag="OH")
                nc.vector.tensor_tensor(
                    out=OH[:], in0=codes[:, c0:c0 + S, None].to_broadcast([128, S, 128]),
                    in1=iota[:, None, :].to_broadcast([128, S, 128]),
                    op=mybir.AluOpType.is_equal)

                # group-level alD / weights / messages
                alps = pal.tile([128, 4 * S], F32, tag="al")
                for cs_ in range(S):
                    t = slot_tile[gi][cs_]
                    nc.tensor.matmul(out=alps[:, 4 * cs_:4 * cs_ + 4], lhsT=OT[:, cs_, :],
                                     rhs=alT1[:, t * 8 + 4:t * 8 + 8], start=True, stop=True)
                alDs = smp.tile([128, S, HEADS], F16, tag="alDs")
                nc.vector.tensor_copy(out=alDs[:], in_=alps[:].rearrange("p (k e) -> p k e", e=4))
                xl = smp.tile([128, S, HEADS], F16, tag="xl")
                nc.vector.tensor_tensor(out=xl[:], in0=G[:, :, 128:132], in1=alDs[:],
                                        op=mybir.AluOpType.add)
                e1 = smp.tile([128, S, HEADS], F16, tag="e1")
                nc.scalar.activation(e1[:], xl[:], mybir.ActivationFunctionType.Exp)
                e2 = smp.tile([128, S, HEADS], F16, tag="e2")
                nc.scalar.activation(e2[:], xl[:], mybir.ActivationFunctionType.Exp, scale=NEG_SLOPE)
                rhs = rp.tile([128, S + nt, 132], F16, tag="rhs1")
                nc.vector.tensor_tensor(out=rhs[:, 0:S, 128:132], in0=e1[:], in1=e2[:],
                                        op=mybir.AluOpType.max)
                nc.vector.tensor_tensor(
                    out=rhs[:, 0:S, 0:128].rearrange("p k (h j) -> p k h j", h=4),
                    in0=G[:, :, 0:128].rearrange("p k (h j) -> p k h j", h=4),
                    in1=rhs[:, 0:S, 128:132][:, :, :, None].to_broadcast([128, S, 4, 32]),
                    op=mybir.AluOpType.mult)
                for ti, t in enumerate(range(t0, t1)):
                    nc.vector.tensor_copy(out=rhs[:, S + ti, 128:132], in_=wself1[:, t, :])
                    nc.vector.tensor_tensor(
                        out=rhs[:, S + ti:S + ti + 1, 0:128].rearrange("p k (h j) -> p k h j", h=4),
                        in0=h_own[:, t * 128:(t + 1) * 128].rearrange("p (h j) -> p h j", h=4)[:, None, :, :],
                        in1=wself1[:, t, None, :, None].to_broadcast([128, 1, 4, 32]),
                        op=mybir.AluOpType.mult)

                for ti, t in enumerate(range(t0, t1)):
                    ca, cb = chA[t], chB[t]
                    slots = list(range(aoff[t], aoff[t] + ca)) + \
                            list(range(SA + boff[t], SA + boff[t] + cb))
                    ps = ppa.tile([128, 132], F32, tag="agg")
                    for ci, cs_ in enumerate(slots):
                        nc.tensor.matmul(out=ps[:], lhsT=OH[:, cs_, :], rhs=rhs[:, cs_, :],
                                         start=(ci == 0), stop=False)
                    nc.tensor.matmul(out=ps[:], lhsT=ident[:], rhs=rhs[:, S + ti, :],
                                     start=False, stop=True)

                    rec = smp.tile([128, HEADS], F32, tag="rec")
                    nc.vector.reciprocal(out=rec[:], in_=ps[:, 128:132])
                    y1 = ep.tile([128, 128], F32, tag="y")
                    nc.vector.tensor_tensor(
                        out=y1[:].rearrange("p (h j) -> p h j", h=4),
                        in0=ps[:, 0:128].rearrange("p (h j) -> p h j", h=4),
                        in1=rec[:, :, None].to_broadcast([128, 4, 32]),
                        op=mybir.AluOpType.mult)
                    nc.vector.tensor_tensor(out=y1[:], in0=y1[:], in1=b1r[:], op=mybir.AluOpType.add)
                    m1 = ep.tile([128, 128], F32, tag="m1")
                    nc.vector.tensor_scalar(out=m1[:], in0=y1[:], scalar1=0.0, scalar2=None,
                                            op0=mybir.AluOpType.min)
                    eE = ep.tile([128, 128], F32, tag="eE")
                    nc.scalar.activation(eE[:], m1[:], mybir.ActivationFunctionType.Exp)
                    r1 = ep.tile([128, 128], F32, tag="r1")
                    nc.vector.tensor_scalar(out=r1[:], in0=y1[:], scalar1=0.0, scalar2=-1.0,
                                            op0=mybir.AluOpType.max, op1=mybir.AluOpType.add)
                    h2 = ep.tile([128, 128], F16, tag="h2")
                    nc.vector.tensor_tensor(out=h2[:], in0=eE[:], in1=r1[:], op=mybir.AluOpType.add)
                    pt = ppm.tile([128, 128], F16, tag="misc")
                    nc.tensor.transpose(out=pt[:], in_=h2[:], identity=ident[:])
                    h2T = ep.tile([128, 128], F16, tag="h2T")
                    nc.vector.tensor_copy(out=h2T[:], in_=pt[:])
                    psz = ppm.tile([128, HID + 2], F32, tag="misc")
                    nc.tensor.matmul(out=psz[:], lhsT=h2T[:], rhs=W2big[:], start=True, stop=True)
                    t2b = sp.tile([128, T2_COLS], F16, tag="t2b")
                    nc.vector.tensor_tensor(out=t2b[:, 0:HID + 2], in0=psz[:], in1=b2r[:],
                                            op=mybir.AluOpType.add)
                    nc.sync.dma_start(out=T2_own[t * 128:(t + 1) * 128, :], in_=t2b[:])
                    nc.vector.tensor_copy(out=z_own[:, t * HID:(t + 1) * HID], in_=t2b[:, 0:HID])
                    nc.vector.tensor_copy(out=alT2[:, t * 2:t * 2 + 2], in_=t2b[:, HID:HID + 2])
                    if t == NT // 2 - 1:
                        allgather(T2_own, T2_sh, 0)
            allgather(T2_own, T2_sh, 1)

            alT2v = alT2[:].rearrange("p (t e) -> p t e", t=NT)
            xls2 = smp.tile([128, NT, 1], F16, tag="xls2")
            nc.vector.tensor_tensor(out=xls2[:], in0=alT2v[:, :, 0:1], in1=alT2v[:, :, 1:2],
                                    op=mybir.AluOpType.add)
            e1s2 = smp.tile([128, NT, 1], F16, tag="e1s2")
            nc.scalar.activation(e1s2[:], xls2[:], mybir.ActivationFunctionType.Exp)
            e2s2 = smp.tile([128, NT, 1], F16, tag="e2s2")
            nc.scalar.activation(e2s2[:], xls2[:], mybir.ActivationFunctionType.Exp, scale=NEG_SLOPE)
            nc.vector.tensor_tensor(out=wself2[:], in0=e1s2[:], in1=e2s2[:], op=mybir.AluOpType.max)

            # ---- layer 2
            for gi, (t0, t1) in enumerate(groups):
                SA, SB = gSA[gi], gSB[gi]
                S = SA + SB
                nt = t1 - t0
                c0 = gc0[gi]
                idxg = ixp.tile([128, S * 8], I16, tag="idx")
                nc.sync.dma_start(out=idxg[:], in_=idx_flat[:, c0 * 8:(c0 + S) * 8])
                G2 = gp.tile([128, S, T2_COLS], F16, tag="G2")
                if SA:
                    nc.gpsimd.dma_gather(
                        G2[:, 0:SA, :], T2_sh[:], idxg[:, 0:SA * 8],
                        128 * SA, 128 * SA, T2_COLS, queue_num=qn(), single_packet=False)
                if SB:
                    nc.gpsimd.dma_gather(
                        G2[:, SA:S, :], T2_sh[b_base:nrows, :], idxg[:, SA * 8:S * 8],
                        128 * SB, 128 * SB, T2_COLS, queue_num=qn(), single_packet=False)
                OT = otp.tile([128, S, 128], F16, tag="OT")
                nc.sync.dma_start(out=OT[:], in_=ohT_d[:, c0:c0 + S, :])
                OH = ohp.tile([128, S, 128], F16, tag="OH")
                nc.vector.tensor_tensor(
                    out=OH[:], in0=codes[:, c0:c0 + S, None].to_broadcast([128, S, 128]),
                    in1=iota[:, None, :].to_broadcast([128, S, 128]),
                    op=mybir.AluOpType.is_equal)

                alps2 = pal.tile([128, S], F32, tag="al")
                for cs_ in range(S):
                    t = slot_tile[gi][cs_]
                    nc.tensor.matmul(out=alps2[:, cs_:cs_ + 1], lhsT=OT[:, cs_, :],
                                     rhs=alT2[:, t * 2 + 1:t * 2 + 2], start=True, stop=True)
                alDs2 = smp.tile([128, S, 1], F16, tag="alDs2")
                nc.vector.tensor_copy(out=alDs2[:], in_=alps2[:, :, None])
                xl2 = smp.tile([128, S, 1], F16, tag="xl2")
                nc.vector.tensor_tensor(out=xl2[:], in0=G2[:, :, 32:33], in1=alDs2[:],
                                        op=mybir.AluOpType.add)
                e1b = smp.tile([128, S, 1], F16, tag="e1b")
                nc.scalar.activation(e1b[:], xl2[:], mybir.ActivationFunctionType.Exp)
                e2b = smp.tile([128, S, 1], F16, tag="e2b")
                nc.scalar.activation(e2b[:], xl2[:], mybir.ActivationFunctionType.Exp, scale=NEG_SLOPE)
                rhs2 = rp.tile([128, S + nt, HID + 1], F16, tag="rhs2")
                nc.vector.tensor_tensor(out=rhs2[:, 0:S, HID:HID + 1], in0=e1b[:], in1=e2b[:],
                                        op=mybir.AluOpType.max)
                nc.vector.tensor_tensor(
                    out=rhs2[:, 0:S, 0:HID], in0=G2[:, :, 0:HID],
                    in1=rhs2[:, 0:S, HID:HID + 1].to_broadcast([128, S, HID]),
                    op=mybir.AluOpType.mult)
                for ti, t in enumerate(range(t0, t1)):
                    nc.vector.tensor_copy(out=rhs2[:, S + ti, HID:HID + 1], in_=wself2[:, t, :])
                    nc.vector.tensor_tensor(
                        out=rhs2[:, S + ti:S + ti + 1, 0:HID],
                        in0=z_own[:, None, t * HID:(t + 1) * HID],
                        in1=wself2[:, t, None, :].to_broadcast([128, 1, HID]),
                        op=mybir.AluOpType.mult)

                for ti, t in enumerate(range(t0, t1)):
                    ca, cb = chA[t], chB[t]
                    slots = list(range(aoff[t], aoff[t] + ca)) + \
                            list(range(SA + boff[t], SA + boff[t] + cb))
                    ps2 = ppa.tile([128, HID + 1], F32, tag="agg")
                    for ci, cs_ in enumerate(slots):
                        nc.tensor.matmul(out=ps2[:], lhsT=OH[:, cs_, :], rhs=rhs2[:, cs_, :],
                                         start=(ci == 0), stop=False)
                    nc.tensor.matmul(out=ps2[:], lhsT=ident[:], rhs=rhs2[:, S + ti, :],
                                     start=False, stop=True)

                    rec2 = smp.tile([128, 1], F32, tag="rec2")
                    nc.vector.reciprocal(out=rec2[:], in_=ps2[:, HID:HID + 1])
                    y2 = ep.tile([128, HID], F32, tag="y2")
                    nc.vector.tensor_tensor(out=y2[:], in0=ps2[:, 0:HID],
                                            in1=rec2[:].to_broadcast([128, HID]),
                                            op=mybir.AluOpType.mult)
                    m2 = ep.tile([128, HID], F32, tag="m2")
                    nc.vector.tensor_scalar(out=m2[:], in0=y2[:], scalar1=0.0, scalar2=None,
                                            op0=mybir.AluOpType.min)
                    eE2 = ep.tile([128, HID], F32, tag="eE2")
                    nc.scalar.activation(eE2[:], m2[:], mybir.ActivationFunctionType.Exp)
                    r2 = ep.tile([128, HID], F32, tag="r2")
                    nc.vector.tensor_scalar(out=r2[:], in0=y2[:], scalar1=0.0, scalar2=-1.0,
                                            op0=mybir.AluOpType.max, op1=mybir.AluOpType.add)
                    h3 = ep.tile([128, HID], F16, tag="h3")
                    nc.vector.tensor_tensor(out=h3[:], in0=eE2[:], in1=r2[:], op=mybir.AluOpType.add)
                    pt2 = ppm.tile([128, 128], F16, tag="misc")
                    nc.tensor.transpose(out=pt2[:HID, :], in_=h3[:], identity=ident[:])
                    h3T = ep.tile([HID, 128], F16, tag="h3T")
                    nc.vector.tensor_copy(out=h3T[:], in_=pt2[:HID, :])
                    psf = ppm.tile([128, OUT_CH], F32, tag="misc")
                    nc.tensor.matmul(out=psf[:], lhsT=h3T[:], rhs=Wout[:], start=True, stop=True)
                    outf = ep.tile([128, OUT_CH], F32, tag="outf")
                    nc.vector.tensor_tensor(out=outf[:], in0=psf[:], in1=boutr[:],
                                            op=mybir.AluOpType.add)
                    nc.sync.dma_start(out=out_d[t * 128:(t + 1) * 128, :], in_=outf[:])

    nc.compile()
    return nc


def _run(inputs, trace=False):
    meta, in_maps = _prep(**inputs)
    nc = _build(meta)
    res = run_bass_kernel_spmd(nc, in_maps, core_ids=list(range(N_CORES)), trace=trace)
    outg = np.concatenate([res.results[c]["out"] for c in range(N_CORES)], axis=0)
    out_nodes = np.empty((meta["n_pad"], OUT_CH), np.float32)
    out_nodes[meta["perm_rows"]] = outg
    return out_nodes[:meta["N"]], res


def kernel(**inputs):
    out, _ = _run(inputs, trace=False)
    return out


# revision 12
# speedup vs baseline: 2.0160x; 1.1505x over previous
"""GAT (2-layer, 4-head then 1-head) on 8 Trainium2 NeuronCores.

Strategy (v3 — dense one-hot chunks, group-level batching)
----------------------------------------------------------
- Nodes degree-sorted and dealt round-robin to 8 cores; each core's 5120
  nodes form 40 dst tiles of 128.
- Edges (self-loops excluded) are packed DENSELY per (tile, table-half)
  into 128-edge chunks (pad ~9%). Aggregation multiplies each chunk by a
  one-hot lhsT whose column d selects the partitions holding edges of
  dst d, accumulating Sum_e w_e*[h_e | 1] per dst in PSUM.
- Per-edge al_dst: alD[p] = onehotT_c (contract dst) al8_own — computed
  for ALL chunks of a group into one PSUM bank, one matmul per chunk.
  onehotT is streamed from DRAM; the aggregation one-hot is generated on
  device (DVE is_equal(codes, iota)).
- All per-edge DVE work (al add, exp, max, message multiply) happens at
  GROUP granularity (~36 chunks per instruction), not per tile; only the
  aggregation matmuls and the epilogue are per tile.
- Self-loops never gathered: extra rhs slots + identity-matmul chunks
  fed from SBUF-resident h_own / z_own.
- Softmax without max-subtraction (exp(lrelu(x)) = max(exp x, exp .2x));
  denominators ride the same one-hot matmul (w columns). Pad slots have
  all-zero one-hot columns.
- Node tables: L1 rows [h(128)|al_src(4)|pad] 512B; L2 rows
  [z+b2(32)|as2(1)|pad] 256B. Table rows are numbered CHUNK-MAJOR
  (pos<2560 first for all cores, then the rest) so each half-table
  AllGather has a contiguous output and can overlap compute. int16
  gather indices via the A/B table split at row 30721.
"""

import numpy as np

import concourse.bacc as bacc
import concourse.mybir as mybir
import concourse.tile as tile
from concourse.bass_utils import run_bass_kernel_spmd

F32 = mybir.dt.float32
F16 = mybir.dt.float16
I16 = mybir.dt.int16

IN_CH = 128
HID = 32
HEADS = 4
OUT_CH = 112
NEG_SLOPE = 0.2

T1_COLS = 256
T2_COLS = 128

N_CORES = 8
GCAP = 36
PAD_CODE = 200
HROWS = 2560  # rows per core per allgather chunk (2 chunks)


def _prep(x, edge_index, W1, a_src1, a_dst1, b1, W2, a_src2, a_dst2, b2, W_out, b_out):
    N = x.shape[0]
    per_core = -(-N // (N_CORES * 128)) * 128
    n_pad = per_core * N_CORES
    NT = per_core // 128
    nrows = n_pad + 2
    b_base = 1 + 6 * per_core
    assert b_base - 1 <= 32767 and nrows - b_base <= 32767
    assert per_core == 2 * HROWS

    src = np.asarray(edge_index[0], np.int64)
    dst = np.asarray(edge_index[1], np.int64)
    E = src.shape[0]

    deg = np.bincount(dst, minlength=n_pad)
    order = np.argsort(deg, kind="stable")
    rank = np.empty(n_pad, np.int64)
    rank[order] = np.arange(n_pad)
    coreid = rank % N_CORES
    pos = rank // N_CORES
    grow = coreid * per_core + pos
    # chunk-major table rows: all cores' pos<HROWS first, then the rest
    trow = 1 + (pos // HROWS) * (N_CORES * HROWS) + coreid * HROWS + pos % HROWS
    perm_rows = np.empty(n_pad, np.int64)
    perm_rows[grow] = np.arange(n_pad)

    sr = trow[src]
    dr = grow[dst]
    gB = sr >= b_base
    core = dr // per_core
    tl = (dr % per_core) // 128
    lane = dr % 128

    EaT = np.zeros((N_CORES, NT), np.int64)
    EbT = np.zeros((N_CORES, NT), np.int64)
    np.add.at(EaT, (core[~gB], tl[~gB]), 1)
    np.add.at(EbT, (core[gB], tl[gB]), 1)
    chA = (-(-EaT // 128)).max(axis=0)
    chB = (-(-EbT // 128)).max(axis=0)

    groups = []
    t = 0
    while t < NT:
        e = t
        tot = 0
        while e < NT and (e == t or tot + chA[e] + chB[e] <= GCAP):
            tot += chA[e] + chB[e]
            e += 1
        groups.append((t, e))
        t = e

    aoff = np.zeros(NT, np.int64)
    boff = np.zeros(NT, np.int64)
    gc0 = []
    gSA = []
    gSB = []
    C = 0
    g_of_tile = np.zeros(NT, np.int64)
    for gi, (t0, t1) in enumerate(groups):
        sa = int(chA[t0:t1].sum())
        sb = int(chB[t0:t1].sum())
        gc0.append(C)
        gSA.append(sa)
        gSB.append(sb)
        off = 0
        for t in range(t0, t1):
            g_of_tile[t] = gi
            aoff[t] = off
            off += chA[t]
        off = 0
        for t in range(t0, t1):
            boff[t] = off
            off += chB[t]
        C += sa + sb
    totidx = C * 128
    assert totidx % 16 == 0

    SENT_A = 0
    SENT_B = nrows - 1 - b_base
    idx_streams = np.zeros((N_CORES, C, 128), np.int16)
    for gi in range(len(groups)):
        idx_streams[:, gc0[gi]:gc0[gi] + gSA[gi], :] = SENT_A
        idx_streams[:, gc0[gi] + gSA[gi]:gc0[gi] + gSA[gi] + gSB[gi], :] = SENT_B
    codes_streams = np.full((N_CORES, C, 128), PAD_CODE, np.int16)

    cbaseA = np.array([gc0[g_of_tile[t]] + aoff[t] for t in range(NT)])
    cbaseB = np.array([gc0[g_of_tile[t]] + gSA[g_of_tile[t]] + boff[t] for t in range(NT)])

    key = (core * NT + tl) * 2 + gB.astype(np.int64)
    eorder = np.argsort(key, kind="stable")
    ks = key[eorder]
    newrun = np.ones(E, bool)
    newrun[1:] = ks[1:] != ks[:-1]
    run_start = np.flatnonzero(newrun)
    run_id = np.cumsum(newrun) - 1
    j = np.arange(E) - run_start[run_id]
    cs = core[eorder]
    tls = tl[eorder]
    gs = gB[eorder]
    cidx = np.where(gs, cbaseB[tls], cbaseA[tls]) + j // 128
    idx_streams[cs, cidx, j % 128] = np.where(gs, sr[eorder] - b_base, sr[eorder]).astype(np.int16)
    codes_streams[cs, cidx, j % 128] = lane[eorder]

    idx_wrapped = np.empty((N_CORES, 128, totidx // 16), np.int16)
    for c in range(N_CORES):
        w16 = idx_streams[c].reshape(-1, 16).T
        idx_wrapped[c] = np.tile(w16, (8, 1))

    f16 = np.float16
    codes_pc = np.transpose(codes_streams, (0, 2, 1)).astype(f16)
    d_ar = np.arange(128, dtype=np.int16)[:, None, None]
    ohT = np.empty((N_CORES, 128, C, 128), f16)
    for c in range(N_CORES):
        ohT[c] = (codes_streams[c][None, :, :] == d_ar).astype(f16)

    xp = np.zeros((n_pad, IN_CH), np.float32)
    xp[:N] = np.asarray(x, np.float32)
    x_slices = np.empty((N_CORES, IN_CH, per_core), np.float32)
    for c in range(N_CORES):
        x_slices[c] = xp[perm_rows[c * per_core:(c + 1) * per_core]].T

    W1 = np.asarray(W1, np.float32)
    Bsrc = np.zeros((HEADS * HID, HEADS), np.float32)
    Bdst = np.zeros((HEADS * HID, HEADS), np.float32)
    for h in range(HEADS):
        Bsrc[h * HID:(h + 1) * HID, h] = np.asarray(a_src1[h], np.float32)
        Bdst[h * HID:(h + 1) * HID, h] = np.asarray(a_dst1[h], np.float32)
    W1big = np.concatenate([W1, W1 @ Bsrc, W1 @ Bdst], axis=1)
    W2 = np.asarray(W2, np.float32)
    W2big = np.concatenate(
        [W2, W2 @ np.asarray(a_src2, np.float32).T, W2 @ np.asarray(a_dst2, np.float32).T],
        axis=1,
    )
    b1_rep = np.tile(np.asarray(b1, np.float32)[None, :], (128, 1))
    b2_rep = np.zeros((128, HID + 2), np.float32)
    b2_rep[:, :HID] = np.asarray(b2, np.float32)[None, :]
    bout_rep = np.tile(np.asarray(b_out, np.float32)[None, :], (128, 1))
    ident = np.eye(128, dtype=f16)
    iota = np.tile(np.arange(128, dtype=f16)[None, :], (128, 1))

    meta = dict(
        N=N, n_pad=n_pad, per_core=per_core, NT=NT, nrows=nrows, b_base=b_base,
        chA=chA.tolist(), chB=chB.tolist(), groups=groups, gc0=gc0, gSA=gSA,
        gSB=gSB, aoff=aoff.tolist(), boff=boff.tolist(), C=C, totidx=totidx,
        perm_rows=perm_rows,
    )
    shared = dict(
        W1big=W1big, W2big=W2big.astype(f16), Wout=np.asarray(W_out, np.float32).astype(f16),
        b1_rep=b1_rep, b2_rep=b2_rep, bout_rep=bout_rep, ident=ident, iota=iota,
    )
    in_maps = []
    for c in range(N_CORES):
        m = dict(shared)
        m["x_slice"] = np.ascontiguousarray(x_slices[c])
        m["idx_flat"] = np.ascontiguousarray(idx_wrapped[c])
        m["codes"] = np.ascontiguousarray(codes_pc[c])
        m["ohT"] = np.ascontiguousarray(ohT[c])
        in_maps.append(m)
    return meta, in_maps


def _build(meta):
    per_core, NT, nrows, b_base = meta["per_core"], meta["NT"], meta["nrows"], meta["b_base"]
    chA, chB = meta["chA"], meta["chB"]
    groups, gc0, gSA, gSB = meta["groups"], meta["gc0"], meta["gSA"], meta["gSB"]
    aoff, boff, C = meta["aoff"], meta["boff"], meta["C"]
    totidx = meta["totidx"]

    nc = bacc.Bacc("TRN2", num_devices=N_CORES, num_swdge_queues=4,
                   dynamic_dma_scratch_size=32768)

    x_slice = nc.dram_tensor("x_slice", [IN_CH, per_core], F32, kind="ExternalInput")
    idx_flat = nc.dram_tensor("idx_flat", [128, totidx // 16], I16, kind="ExternalInput")
    codes_d = nc.dram_tensor("codes", [128, C], F16, kind="ExternalInput")
    ohT_d = nc.dram_tensor("ohT", [128, C, 128], F16, kind="ExternalInput")
    W1big_d = nc.dram_tensor("W1big", [128, 136], F32, kind="ExternalInput")
    W2big_d = nc.dram_tensor("W2big", [128, HID + 2], F16, kind="ExternalInput")
    Wout_d = nc.dram_tensor("Wout", [HID, OUT_CH], F16, kind="ExternalInput")
    b1_d = nc.dram_tensor("b1_rep", [128, 128], F32, kind="ExternalInput")
    b2_d = nc.dram_tensor("b2_rep", [128, HID + 2], F32, kind="ExternalInput")
    bout_d = nc.dram_tensor("bout_rep", [128, OUT_CH], F32, kind="ExternalInput")
    ident_d = nc.dram_tensor("ident", [128, 128], F16, kind="ExternalInput")
    iota_d = nc.dram_tensor("iota", [128, 128], F16, kind="ExternalInput")

    T1_own = nc.dram_tensor("T1_own", [per_core, T1_COLS], F16, kind="Internal")
    T1_sh = nc.dram_tensor("T1_sh", [nrows, T1_COLS], F16, kind="Internal", addr_space="Shared")
    T2_own = nc.dram_tensor("T2_own", [per_core, T2_COLS], F16, kind="Internal")
    T2_sh = nc.dram_tensor("T2_sh", [nrows, T2_COLS], F16, kind="Internal", addr_space="Shared")
    out_d = nc.dram_tensor("out", [per_core, OUT_CH], F32, kind="ExternalOutput")

    rgroups = [list(range(N_CORES))]
    qctr = [0]

    def qn():
        q = qctr[0] % 4
        qctr[0] += 1
        return q

    def allgather(own, sh, half):
        r0 = half * HROWS
        o0 = 1 + half * N_CORES * HROWS
        nc.gpsimd.collective_compute(
            "AllGather", mybir.AluOpType.bypass, replica_groups=rgroups,
            ins=[own[r0:r0 + HROWS, :]], outs=[sh[o0:o0 + N_CORES * HROWS, :]],
        )

    # per-group slot -> tile map
    slot_tile = []
    for gi, (t0, t1) in enumerate(groups):
        st = []
        for t in range(t0, t1):
            st += [t] * chA[t]
        for t in range(t0, t1):
            st += [t] * chB[t]
        slot_tile.append(st)

    with tile.TileContext(nc) as tc:
        with (
            tc.tile_pool(name="const", bufs=1) as cp,
            tc.tile_pool(name="persist", bufs=1) as pp,
            tc.tile_pool(name="xa", bufs=2) as xap,
            tc.tile_pool(name="stage", bufs=3) as sp,
            tc.tile_pool(name="idxp", bufs=3) as ixp,
            tc.tile_pool(name="gath", bufs=3) as gp,
            tc.tile_pool(name="ot", bufs=2) as otp,
            tc.tile_pool(name="oh", bufs=2) as ohp,
            tc.tile_pool(name="rhs", bufs=2) as rp,
            tc.tile_pool(name="small", bufs=3) as smp,
            tc.tile_pool(name="epi", bufs=2) as ep,
            tc.tile_pool(name="psa", bufs=3, space="PSUM") as ppa,
            tc.tile_pool(name="psal", bufs=2, space="PSUM") as pal,
            tc.tile_pool(name="psm", bufs=3, space="PSUM") as ppm,
        ):
            # ---- consts
            W1big = cp.tile([128, 136], F32)
            nc.sync.dma_start(out=W1big[:], in_=W1big_d[:])
            W2big = cp.tile([128, HID + 2], F16)
            nc.sync.dma_start(out=W2big[:], in_=W2big_d[:])
            Wout = cp.tile([HID, OUT_CH], F16)
            nc.sync.dma_start(out=Wout[:], in_=Wout_d[:])
            b1r = cp.tile([128, 128], F32)
            nc.sync.dma_start(out=b1r[:], in_=b1_d[:])
            b2r = cp.tile([128, HID + 2], F32)
            nc.sync.dma_start(out=b2r[:], in_=b2_d[:])
            boutr = cp.tile([128, OUT_CH], F32)
            nc.sync.dma_start(out=boutr[:], in_=bout_d[:])
            ident = cp.tile([128, 128], F16)
            nc.sync.dma_start(out=ident[:], in_=ident_d[:])
            iota = cp.tile([128, 128], F16)
            nc.sync.dma_start(out=iota[:], in_=iota_d[:])
            codes = cp.tile([128, C], F16)
            nc.sync.dma_start(out=codes[:], in_=codes_d[:])

            h_own = pp.tile([128, NT * 128], F16)
            alT1 = pp.tile([128, NT * 8], F16)
            z_own = pp.tile([128, NT * HID], F16)
            alT2 = pp.tile([128, NT * 2], F16)
            wself1 = pp.tile([128, NT, HEADS], F16)
            wself2 = pp.tile([128, NT, 1], F16)

            zs1 = cp.tile([1, T1_COLS], F16)
            nc.vector.memset(zs1[:], 0.0)
            nc.sync.dma_start(out=T1_sh[0:1, :], in_=zs1[:])
            nc.sync.dma_start(out=T1_sh[nrows - 1:nrows, :], in_=zs1[:])
            nc.sync.dma_start(out=T2_sh[0:1, :], in_=zs1[:, 0:T2_COLS])
            nc.sync.dma_start(out=T2_sh[nrows - 1:nrows, :], in_=zs1[:, 0:T2_COLS])

            # ---- phase A
            for t in range(NT):
                xa = xap.tile([128, 128], F32)
                nc.sync.dma_start(out=xa[:], in_=x_slice[:, t * 128:(t + 1) * 128])
                ps = ppa.tile([128, 136], F32, tag="agg")
                nc.tensor.matmul(out=ps[:], lhsT=xa[:], rhs=W1big[:], start=True, stop=True)
                hb = sp.tile([128, T1_COLS], F16, tag="hb")
                nc.vector.tensor_copy(out=hb[:, 0:132], in_=ps[:, 0:132])
                nc.sync.dma_start(out=T1_own[t * 128:(t + 1) * 128, :], in_=hb[:])
                nc.vector.tensor_copy(out=h_own[:, t * 128:(t + 1) * 128], in_=ps[:, 0:128])
                nc.vector.tensor_copy(out=alT1[:, t * 8:t * 8 + 8], in_=ps[:, 128:136])
                if t == NT // 2 - 1:
                    allgather(T1_own, T1_sh, 0)
            allgather(T1_own, T1_sh, 1)

            alT1v = alT1[:].rearrange("p (t e) -> p t e", t=NT)
            xls = smp.tile([128, NT, HEADS], F16, tag="xls")
            nc.vector.tensor_tensor(out=xls[:], in0=alT1v[:, :, 0:4], in1=alT1v[:, :, 4:8],
                                    op=mybir.AluOpType.add)
            e1s = smp.tile([128, NT, HEADS], F16, tag="e1s")
            nc.scalar.activation(e1s[:], xls[:], mybir.ActivationFunctionType.Exp)
            e2s = smp.tile([128, NT, HEADS], F16, tag="e2s")
            nc.scalar.activation(e2s[:], xls[:], mybir.ActivationFunctionType.Exp, scale=NEG_SLOPE)
            nc.vector.tensor_tensor(out=wself1[:], in0=e1s[:], in1=e2s[:], op=mybir.AluOpType.max)

            # ---- layer 1
            for gi, (t0, t1) in enumerate(groups):
                SA, SB = gSA[gi], gSB[gi]
                S = SA + SB
                nt = t1 - t0
                c0 = gc0[gi]
                idxg = ixp.tile([128, S * 8], I16, tag="idx")
                nc.sync.dma_start(out=idxg[:], in_=idx_flat[:, c0 * 8:(c0 + S) * 8])
                G = gp.tile([128, S, T1_COLS], F16, tag="G1")
                if SA:
                    nc.gpsimd.dma_gather(
                        G[:, 0:SA, :], T1_sh[:], idxg[:, 0:SA * 8],
                        128 * SA, 128 * SA, T1_COLS, queue_num=qn(), single_packet=False)
                if SB:
                    nc.gpsimd.dma_gather(
                        G[:, SA:S, :], T1_sh[b_base:nrows, :], idxg[:, SA * 8:S * 8],
                        128 * SB, 128 * SB, T1_COLS, queue_num=qn(), single_packet=False)
                OT = otp.tile([128, S, 128], F16, tag="OT")
                nc.sync.dma_start(out=OT[:], in_=ohT_d[:, c0:c0 + S, :])
                OH = ohp.tile([128, S, 128], F16, tag="OH")
                nc.vector.tensor_tensor(
                    out=OH[:], in0=codes[:, c0:c0 + S, None].to_broadcast([128, S, 128]),
                    in1=iota[:, None, :].to_broadcast([128, S, 128]),
                    op=mybir.AluOpType.is_equal)

                # group-level alD / weights / messages
                alps = pal.tile([128, 4 * S], F32, tag="al")
                for cs_ in range(S):
                    t = slot_tile[gi][cs_]
                    nc.tensor.matmul(out=alps[:, 4 * cs_:4 * cs_ + 4], lhsT=OT[:, cs_, :],
                                     rhs=alT1[:, t * 8 + 4:t * 8 + 8], start=True, stop=True)
                alDs = smp.tile([128, S, HEADS], F16, tag="alDs")
                nc.vector.tensor_copy(out=alDs[:], in_=alps[:].rearrange("p (k e) -> p k e", e=4))
                xl = smp.tile([128, S, HEADS], F16, tag="xl")
                nc.vector.tensor_tensor(out=xl[:], in0=G[:, :, 128:132], in1=alDs[:],
                                        op=mybir.AluOpType.add)
                e1 = smp.tile([128, S, HEADS], F16, tag="e1")
                nc.scalar.activation(e1[:], xl[:], mybir.ActivationFunctionType.Exp)
                e2 = smp.tile([128, S, HEADS], F16, tag="e2")
                nc.scalar.activation(e2[:], xl[:], mybir.ActivationFunctionType.Exp, scale=NEG_SLOPE)
                rhs = rp.tile([128, S + nt, 132], F16, tag="rhs1")
                nc.vector.tensor_tensor(out=rhs[:, 0:S, 128:132], in0=e1[:], in1=e2[:],
                                        op=mybir.AluOpType.max)
                nc.vector.tensor_tensor(
                    out=rhs[:, 0:S, 0:128].rearrange("p k (h j) -> p k h j", h=4),
                    in0=G[:, :, 0:128].rearrange("p k (h j) -> p k h j", h=4),
                    in1=rhs[:, 0:S, 128:132][:, :, :, None].to_broadcast([128, S, 4, 32]),
                    op=mybir.AluOpType.mult)
                nc.vector.tensor_copy(
                    out=rhs[:, S:S + nt, 128:132], in_=wself1[:, t0:t1, :])
                nc.vector.tensor_tensor(
                    out=rhs[:, S:S + nt, 0:128].rearrange("p k (h j) -> p k h j", h=4),
                    in0=h_own[:, t0 * 128:t1 * 128].rearrange("p (t h j) -> p t h j", t=nt, h=4),
                    in1=wself1[:, t0:t1, :, None].to_broadcast([128, nt, 4, 32]),
                    op=mybir.AluOpType.mult)

                psall = ep.tile([128, nt, 132], F32, tag="psall")
                for ti, t in enumerate(range(t0, t1)):
                    ca, cb = chA[t], chB[t]
                    slots = list(range(aoff[t], aoff[t] + ca)) + \
                            list(range(SA + boff[t], SA + boff[t] + cb))
                    ps = ppa.tile([128, 132], F32, tag="agg")
                    for ci, cs_ in enumerate(slots):
                        nc.tensor.matmul(out=ps[:], lhsT=OH[:, cs_, :], rhs=rhs[:, cs_, :],
                                         start=(ci == 0), stop=False)
                    nc.tensor.matmul(out=ps[:], lhsT=ident[:], rhs=rhs[:, S + ti, :],
                                     start=False, stop=True)
                    nc.vector.tensor_copy(out=psall[:, ti, :], in_=ps[:])
                psg_v = psall[:]

                # group-level epilogue
                rec = smp.tile([128, nt, HEADS], F32, tag="rec")
                nc.vector.reciprocal(out=rec[:], in_=psg_v[:, :, 128:132])
                y1 = ep.tile([128, nt, 128], F16, tag="y")
                nc.vector.tensor_tensor(
                    out=y1[:].rearrange("p t (h j) -> p t h j", h=4),
                    in0=psg_v[:, :, 0:128].rearrange("p t (h j) -> p t h j", h=4),
                    in1=rec[:, :, :, None].to_broadcast([128, nt, 4, 32]),
                    op=mybir.AluOpType.mult)
                nc.vector.tensor_tensor(
                    out=y1[:], in0=y1[:],
                    in1=b1r[:, None, :].to_broadcast([128, nt, 128]),
                    op=mybir.AluOpType.add)
                m1 = ep.tile([128, nt, 128], F16, tag="m1")
                nc.vector.tensor_scalar(out=m1[:], in0=y1[:], scalar1=0.0, scalar2=None,
                                        op0=mybir.AluOpType.min)
                nc.scalar.activation(m1[:], m1[:], mybir.ActivationFunctionType.Exp)
                nc.vector.tensor_scalar(out=y1[:], in0=y1[:], scalar1=0.0, scalar2=-1.0,
                                        op0=mybir.AluOpType.max, op1=mybir.AluOpType.add)
                h2 = m1
                nc.vector.tensor_tensor(out=h2[:], in0=m1[:], in1=y1[:], op=mybir.AluOpType.add)

                t2g = sp.tile([128, nt, T2_COLS], F16, tag="t2b")
                for ti, t in enumerate(range(t0, t1)):
                    pt = ppm.tile([128, 128], F16, tag="misc")
                    nc.tensor.transpose(out=pt[:], in_=h2[:, ti, :], identity=ident[:])
                    h2T = ep.tile([128, 128], F16, tag="h2T")
                    nc.vector.tensor_copy(out=h2T[:], in_=pt[:])
                    psz = ppm.tile([128, HID + 2], F32, tag="misc")
                    nc.tensor.matmul(out=psz[:], lhsT=h2T[:], rhs=W2big[:], start=True, stop=True)
                    nc.vector.tensor_tensor(out=t2g[:, ti, 0:HID + 2], in0=psz[:], in1=b2r[:],
                                            op=mybir.AluOpType.add)
                nc.sync.dma_start(
                    out=T2_own[t0 * 128:t1 * 128, :].rearrange("(t p) c -> p t c", p=128),
                    in_=t2g[:])
                nc.vector.tensor_copy(
                    out=z_own[:, t0 * HID:t1 * HID].rearrange("p (t c) -> p t c", t=nt),
                    in_=t2g[:, :, 0:HID])
                nc.vector.tensor_copy(
                    out=alT2[:, t0 * 2:t1 * 2].rearrange("p (t c) -> p t c", t=nt),
                    in_=t2g[:, :, HID:HID + 2])
                if t0 < NT // 2 <= t1:
                    allgather(T2_own, T2_sh, 0)
            allgather(T2_own, T2_sh, 1)

            alT2v = alT2[:].rearrange("p (t e) -> p t e", t=NT)
            xls2 = smp.tile([128, NT, 1], F16, tag="xls2")
            nc.vector.tensor_tensor(out=xls2[:], in0=alT2v[:, :, 0:1], in1=alT2v[:, :, 1:2],
                                    op=mybir.AluOpType.add)
            e1s2 = smp.tile([128, NT, 1], F16, tag="e1s2")
            nc.scalar.activation(e1s2[:], xls2[:], mybir.ActivationFunctionType.Exp)
            e2s2 = smp.tile([128, NT, 1], F16, tag="e2s2")
            nc.scalar.activation(e2s2[:], xls2[:], mybir.ActivationFunctionType.Exp, scale=NEG_SLOPE)
            nc.vector.tensor_tensor(out=wself2[:], in0=e1s2[:], in1=e2s2[:], op=mybir.AluOpType.max)

            # ---- layer 2
            for gi, (t0, t1) in enumerate(groups):
                SA, SB = gSA[gi], gSB[gi]
                S = SA + SB
                nt = t1 - t0
                c0 = gc0[gi]
                idxg = ixp.tile([128, S * 8], I16, tag="idx")
                nc.sync.dma_start(out=idxg[:], in_=idx_flat[:, c0 * 8:(c0 + S) * 8])
                G2 = gp.tile([128, S, T2_COLS], F16, tag="G2")
                if SA:
                    nc.gpsimd.dma_gather(
                        G2[:, 0:SA, :], T2_sh[:], idxg[:, 0:SA * 8],
                        128 * SA, 128 * SA, T2_COLS, queue_num=qn(), single_packet=False)
                if SB:
                    nc.gpsimd.dma_gather(
                        G2[:, SA:S, :], T2_sh[b_base:nrows, :], idxg[:, SA * 8:S * 8],
                        128 * SB, 128 * SB, T2_COLS, queue_num=qn(), single_packet=False)
                OT = otp.tile([128, S, 128], F16, tag="OT")
                nc.sync.dma_start(out=OT[:], in_=ohT_d[:, c0:c0 + S, :])
                OH = ohp.tile([128, S, 128], F16, tag="OH")
                nc.vector.tensor_tensor(
                    out=OH[:], in0=codes[:, c0:c0 + S, None].to_broadcast([128, S, 128]),
                    in1=iota[:, None, :].to_broadcast([128, S, 128]),
                    op=mybir.AluOpType.is_equal)

                alps2 = pal.tile([128, S], F32, tag="al")
                for cs_ in range(S):
                    t = slot_tile[gi][cs_]
                    nc.tensor.matmul(out=alps2[:, cs_:cs_ + 1], lhsT=OT[:, cs_, :],
                                     rhs=alT2[:, t * 2 + 1:t * 2 + 2], start=True, stop=True)
                alDs2 = smp.tile([128, S, 1], F16, tag="alDs2")
                nc.vector.tensor_copy(out=alDs2[:], in_=alps2[:, :, None])
                xl2 = smp.tile([128, S, 1], F16, tag="xl2")
                nc.vector.tensor_tensor(out=xl2[:], in0=G2[:, :, 32:33], in1=alDs2[:],
                                        op=mybir.AluOpType.add)
                e1b = smp.tile([128, S, 1], F16, tag="e1b")
                nc.scalar.activation(e1b[:], xl2[:], mybir.ActivationFunctionType.Exp)
                e2b = smp.tile([128, S, 1], F16, tag="e2b")
                nc.scalar.activation(e2b[:], xl2[:], mybir.ActivationFunctionType.Exp, scale=NEG_SLOPE)
                rhs2 = rp.tile([128, S + nt, HID + 1], F16, tag="rhs2")
                nc.vector.tensor_tensor(out=rhs2[:, 0:S, HID:HID + 1], in0=e1b[:], in1=e2b[:],
                                        op=mybir.AluOpType.max)
                nc.vector.tensor_tensor(
                    out=rhs2[:, 0:S, 0:HID], in0=G2[:, :, 0:HID],
                    in1=rhs2[:, 0:S, HID:HID + 1].to_broadcast([128, S, HID]),
                    op=mybir.AluOpType.mult)
                nc.vector.tensor_copy(
                    out=rhs2[:, S:S + nt, HID:HID + 1], in_=wself2[:, t0:t1, :])
                nc.vector.tensor_tensor(
                    out=rhs2[:, S:S + nt, 0:HID],
                    in0=z_own[:, t0 * HID:t1 * HID].rearrange("p (t c) -> p t c", t=nt),
                    in1=wself2[:, t0:t1, :].to_broadcast([128, nt, HID]),
                    op=mybir.AluOpType.mult)

                psall2 = ep.tile([128, nt, 33], F32, tag="psall2")
                for ti, t in enumerate(range(t0, t1)):
                    ca, cb = chA[t], chB[t]
                    slots = list(range(aoff[t], aoff[t] + ca)) + \
                            list(range(SA + boff[t], SA + boff[t] + cb))
                    ps2 = ppa.tile([128, 33], F32, tag="agg")
                    for ci, cs_ in enumerate(slots):
                        nc.tensor.matmul(out=ps2[:], lhsT=OH[:, cs_, :], rhs=rhs2[:, cs_, :],
                                         start=(ci == 0), stop=False)
                    nc.tensor.matmul(out=ps2[:], lhsT=ident[:], rhs=rhs2[:, S + ti, :],
                                     start=False, stop=True)
                    nc.vector.tensor_copy(out=psall2[:, ti, :], in_=ps2[:])
                psg2_v = psall2[:]

                rec2 = smp.tile([128, nt, 1], F32, tag="rec2")
                nc.vector.reciprocal(out=rec2[:], in_=psg2_v[:, :, HID:HID + 1])
                y2 = ep.tile([128, nt, HID], F16, tag="y2")
                nc.vector.tensor_tensor(out=y2[:], in0=psg2_v[:, :, 0:HID],
                                        in1=rec2[:].to_broadcast([128, nt, HID]),
                                        op=mybir.AluOpType.mult)
                m2 = ep.tile([128, nt, HID], F16, tag="m2")
                nc.vector.tensor_scalar(out=m2[:], in0=y2[:], scalar1=0.0, scalar2=None,
                                        op0=mybir.AluOpType.min)
                nc.scalar.activation(m2[:], m2[:], mybir.ActivationFunctionType.Exp)
                nc.vector.tensor_scalar(out=y2[:], in0=y2[:], scalar1=0.0, scalar2=-1.0,
                                        op0=mybir.AluOpType.max, op1=mybir.AluOpType.add)
                h3 = m2
                nc.vector.tensor_tensor(out=h3[:], in0=m2[:], in1=y2[:], op=mybir.AluOpType.add)

                outg = ep.tile([128, nt, OUT_CH], F32, tag="outf")
                for ti, t in enumerate(range(t0, t1)):
                    pt2 = ppm.tile([128, 128], F16, tag="misc")
                    nc.tensor.transpose(out=pt2[:HID, :], in_=h3[:, ti, :], identity=ident[:])
                    h3T = ep.tile([HID, 128], F16, tag="h3T")
                    nc.vector.tensor_copy(out=h3T[:], in_=pt2[:HID, :])
                    psf = ppm.tile([128, OUT_CH], F32, tag="misc")
                    nc.tensor.matmul(out=psf[:], lhsT=h3T[:], rhs=Wout[:], start=True, stop=True)
                    nc.vector.tensor_tensor(out=outg[:, ti, :], in0=psf[:], in1=boutr[:],
                                            op=mybir.AluOpType.add)
                nc.sync.dma_start(
                    out=out_d[t0 * 128:t1 * 128, :].rearrange("(t p) c -> p t c", p=128),
                    in_=outg[:])

    nc.compile()
    return nc


def _run(inputs, trace=False):
    meta, in_maps = _prep(**inputs)
    nc = _build(meta)
    res = run_bass_kernel_spmd(nc, in_maps, core_ids=list(range(N_CORES)), trace=trace)
    outg = np.concatenate([res.results[c]["out"] for c in range(N_CORES)], axis=0)
    out_nodes = np.empty((meta["n_pad"], OUT_CH), np.float32)
    out_nodes[meta["perm_rows"]] = outg
    return out_nodes[:meta["N"]], res


def kernel(**inputs):
    out, _ = _run(inputs, trace=False)
    return out


# revision 13
# speedup vs baseline: 2.3751x; 1.1781x over previous
"""GAT (2-layer, 4-head then 1-head) on 8 Trainium2 NeuronCores.

Strategy (v3 — dense one-hot chunks, group-level batching)
----------------------------------------------------------
- Nodes degree-sorted and dealt round-robin to 8 cores; each core's 5120
  nodes form 40 dst tiles of 128.
- Edges (self-loops excluded) are packed DENSELY per (tile, table-half)
  into 128-edge chunks (pad ~9%). Aggregation multiplies each chunk by a
  one-hot lhsT whose column d selects the partitions holding edges of
  dst d, accumulating Sum_e w_e*[h_e | 1] per dst in PSUM.
- Per-edge al_dst: alD[p] = onehotT_c (contract dst) al8_own — computed
  for ALL chunks of a group into one PSUM bank, one matmul per chunk.
  onehotT is streamed from DRAM; the aggregation one-hot is generated on
  device (DVE is_equal(codes, iota)).
- All per-edge DVE work (al add, exp, max, message multiply) happens at
  GROUP granularity (~36 chunks per instruction), not per tile; only the
  aggregation matmuls and the epilogue are per tile.
- Self-loops never gathered: extra rhs slots + identity-matmul chunks
  fed from SBUF-resident h_own / z_own.
- Softmax without max-subtraction (exp(lrelu(x)) = max(exp x, exp .2x));
  denominators ride the same one-hot matmul (w columns). Pad slots have
  all-zero one-hot columns.
- Node tables: L1 rows [h(128)|al_src(4)|pad] 512B; L2 rows
  [z+b2(32)|as2(1)|pad] 256B. Table rows are numbered CHUNK-MAJOR
  (pos<2560 first for all cores, then the rest) so each half-table
  AllGather has a contiguous output and can overlap compute. int16
  gather indices via the A/B table split at row 30721.
"""

import numpy as np

import concourse.bacc as bacc
import concourse.mybir as mybir
import concourse.tile as tile
from concourse.bass_utils import run_bass_kernel_spmd

F32 = mybir.dt.float32
F16 = mybir.dt.float16
I16 = mybir.dt.int16

IN_CH = 128
HID = 32
HEADS = 4
OUT_CH = 112
NEG_SLOPE = 0.2

T1_COLS = 256
T2_COLS = 128

N_CORES = 8
GCAP = 36
PAD_CODE = 200
HROWS = 2560  # rows per core per allgather chunk (2 chunks)


def _prep(x, edge_index, W1, a_src1, a_dst1, b1, W2, a_src2, a_dst2, b2, W_out, b_out):
    N = x.shape[0]
    per_core = -(-N // (N_CORES * 128)) * 128
    n_pad = per_core * N_CORES
    NT = per_core // 128
    nrows = n_pad + 2
    b_base = 1 + N_CORES * HROWS
    assert b_base - 1 <= 32767 and nrows - b_base <= 32767
    assert per_core == 2 * HROWS

    src = np.asarray(edge_index[0], np.int64)
    dst = np.asarray(edge_index[1], np.int64)
    E = src.shape[0]

    deg = np.bincount(dst, minlength=n_pad)
    order = np.argsort(deg, kind="stable")
    rank = np.empty(n_pad, np.int64)
    rank[order] = np.arange(n_pad)
    coreid = rank % N_CORES
    pos = rank // N_CORES
    grow = coreid * per_core + pos
    # chunk-major table rows: all cores' pos<HROWS first, then the rest
    trow = 1 + (pos // HROWS) * (N_CORES * HROWS) + coreid * HROWS + pos % HROWS
    perm_rows = np.empty(n_pad, np.int64)
    perm_rows[grow] = np.arange(n_pad)

    sr = trow[src]
    dr = grow[dst]
    gB = sr >= b_base
    core = dr // per_core
    tl = (dr % per_core) // 128
    lane = dr % 128

    EaT = np.zeros((N_CORES, NT), np.int64)
    EbT = np.zeros((N_CORES, NT), np.int64)
    np.add.at(EaT, (core[~gB], tl[~gB]), 1)
    np.add.at(EbT, (core[gB], tl[gB]), 1)
    chA = (-(-EaT // 128)).max(axis=0)
    chB = (-(-EbT // 128)).max(axis=0)

    groups = []
    t = 0
    while t < NT:
        e = t
        tot = 0
        while e < NT and (e == t or tot + chA[e] + chB[e] <= GCAP):
            tot += chA[e] + chB[e]
            e += 1
        groups.append((t, e))
        t = e

    aoff = np.zeros(NT, np.int64)
    boff = np.zeros(NT, np.int64)
    gc0 = []
    gSA = []
    gSB = []
    C = 0
    g_of_tile = np.zeros(NT, np.int64)
    for gi, (t0, t1) in enumerate(groups):
        sa = int(chA[t0:t1].sum())
        sb = int(chB[t0:t1].sum())
        gc0.append(C)
        gSA.append(sa)
        gSB.append(sb)
        off = 0
        for t in range(t0, t1):
            g_of_tile[t] = gi
            aoff[t] = off
            off += chA[t]
        off = 0
        for t in range(t0, t1):
            boff[t] = off
            off += chB[t]
        C += sa + sb
    totidx = C * 128
    assert totidx % 16 == 0

    SENT_A = 0
    SENT_B = nrows - 1 - b_base
    idx_streams = np.zeros((N_CORES, C, 128), np.int16)
    for gi in range(len(groups)):
        idx_streams[:, gc0[gi]:gc0[gi] + gSA[gi], :] = SENT_A
        idx_streams[:, gc0[gi] + gSA[gi]:gc0[gi] + gSA[gi] + gSB[gi], :] = SENT_B
    codes_streams = np.full((N_CORES, C, 128), PAD_CODE, np.int16)

    cbaseA = np.array([gc0[g_of_tile[t]] + aoff[t] for t in range(NT)])
    cbaseB = np.array([gc0[g_of_tile[t]] + gSA[g_of_tile[t]] + boff[t] for t in range(NT)])

    key = (core * NT + tl) * 2 + gB.astype(np.int64)
    eorder = np.argsort(key, kind="stable")
    ks = key[eorder]
    newrun = np.ones(E, bool)
    newrun[1:] = ks[1:] != ks[:-1]
    run_start = np.flatnonzero(newrun)
    run_id = np.cumsum(newrun) - 1
    j = np.arange(E) - run_start[run_id]
    cs = core[eorder]
    tls = tl[eorder]
    gs = gB[eorder]
    cidx = np.where(gs, cbaseB[tls], cbaseA[tls]) + j // 128
    idx_streams[cs, cidx, j % 128] = np.where(gs, sr[eorder] - b_base, sr[eorder]).astype(np.int16)
    codes_streams[cs, cidx, j % 128] = lane[eorder]

    idx_wrapped = np.empty((N_CORES, 128, totidx // 16), np.int16)
    for c in range(N_CORES):
        w16 = idx_streams[c].reshape(-1, 16).T
        idx_wrapped[c] = np.tile(w16, (8, 1))

    f16 = np.float16
    codes_pc = np.transpose(codes_streams, (0, 2, 1)).astype(f16)
    d_ar = np.arange(128, dtype=np.int16)[:, None, None]
    ohT = np.empty((N_CORES, 128, C, 128), f16)
    for c in range(N_CORES):
        ohT[c] = (codes_streams[c][None, :, :] == d_ar).astype(f16)

    xp = np.zeros((n_pad, IN_CH), np.float32)
    xp[:N] = np.asarray(x, np.float32)
    x_slices = np.empty((N_CORES, IN_CH, per_core), np.float32)
    for c in range(N_CORES):
        x_slices[c] = xp[perm_rows[c * per_core:(c + 1) * per_core]].T

    W1 = np.asarray(W1, np.float32)
    Bsrc = np.zeros((HEADS * HID, HEADS), np.float32)
    Bdst = np.zeros((HEADS * HID, HEADS), np.float32)
    for h in range(HEADS):
        Bsrc[h * HID:(h + 1) * HID, h] = np.asarray(a_src1[h], np.float32)
        Bdst[h * HID:(h + 1) * HID, h] = np.asarray(a_dst1[h], np.float32)
    W1big = np.concatenate([W1, W1 @ Bsrc, W1 @ Bdst], axis=1)
    W2 = np.asarray(W2, np.float32)
    W2big = np.concatenate(
        [W2, W2 @ np.asarray(a_src2, np.float32).T, W2 @ np.asarray(a_dst2, np.float32).T],
        axis=1,
    )
    b1_rep = np.tile(np.asarray(b1, np.float32)[None, :], (128, 1))
    b2_rep = np.zeros((128, HID + 2), np.float32)
    b2_rep[:, :HID] = np.asarray(b2, np.float32)[None, :]
    bout_rep = np.tile(np.asarray(b_out, np.float32)[None, :], (128, 1))
    ident = np.eye(128, dtype=f16)
    iota = np.tile(np.arange(128, dtype=f16)[None, :], (128, 1))

    meta = dict(
        N=N, n_pad=n_pad, per_core=per_core, NT=NT, nrows=nrows, b_base=b_base,
        chA=chA.tolist(), chB=chB.tolist(), groups=groups, gc0=gc0, gSA=gSA,
        gSB=gSB, aoff=aoff.tolist(), boff=boff.tolist(), C=C, totidx=totidx,
        perm_rows=perm_rows,
    )
    shared = dict(
        W1big=W1big, W2big=W2big.astype(f16), Wout=np.asarray(W_out, np.float32).astype(f16),
        b1_rep=b1_rep, b2_rep=b2_rep, bout_rep=bout_rep, ident=ident, iota=iota,
    )
    in_maps = []
    for c in range(N_CORES):
        m = dict(shared)
        m["x_slice"] = np.ascontiguousarray(x_slices[c])
        m["idx_flat"] = np.ascontiguousarray(idx_wrapped[c])
        m["codes"] = np.ascontiguousarray(codes_pc[c])
        m["ohT"] = np.ascontiguousarray(ohT[c])
        in_maps.append(m)
    return meta, in_maps


def _build(meta):
    per_core, NT, nrows, b_base = meta["per_core"], meta["NT"], meta["nrows"], meta["b_base"]
    chA, chB = meta["chA"], meta["chB"]
    groups, gc0, gSA, gSB = meta["groups"], meta["gc0"], meta["gSA"], meta["gSB"]
    aoff, boff, C = meta["aoff"], meta["boff"], meta["C"]
    totidx = meta["totidx"]

    nc = bacc.Bacc("TRN2", num_devices=N_CORES, num_swdge_queues=4,
                   dynamic_dma_scratch_size=32768)

    x_slice = nc.dram_tensor("x_slice", [IN_CH, per_core], F32, kind="ExternalInput")
    idx_flat = nc.dram_tensor("idx_flat", [128, totidx // 16], I16, kind="ExternalInput")
    codes_d = nc.dram_tensor("codes", [128, C], F16, kind="ExternalInput")
    ohT_d = nc.dram_tensor("ohT", [128, C, 128], F16, kind="ExternalInput")
    W1big_d = nc.dram_tensor("W1big", [128, 136], F32, kind="ExternalInput")
    W2big_d = nc.dram_tensor("W2big", [128, HID + 2], F16, kind="ExternalInput")
    Wout_d = nc.dram_tensor("Wout", [HID, OUT_CH], F16, kind="ExternalInput")
    b1_d = nc.dram_tensor("b1_rep", [128, 128], F32, kind="ExternalInput")
    b2_d = nc.dram_tensor("b2_rep", [128, HID + 2], F32, kind="ExternalInput")
    bout_d = nc.dram_tensor("bout_rep", [128, OUT_CH], F32, kind="ExternalInput")
    ident_d = nc.dram_tensor("ident", [128, 128], F16, kind="ExternalInput")
    iota_d = nc.dram_tensor("iota", [128, 128], F16, kind="ExternalInput")

    T1_own = nc.dram_tensor("T1_own", [per_core, T1_COLS], F16, kind="Internal")
    T1_sh = nc.dram_tensor("T1_sh", [nrows, T1_COLS], F16, kind="Internal", addr_space="Shared")
    T2_own = nc.dram_tensor("T2_own", [per_core, T2_COLS], F16, kind="Internal")
    T2_sh = nc.dram_tensor("T2_sh", [nrows, T2_COLS], F16, kind="Internal", addr_space="Shared")
    out_d = nc.dram_tensor("out", [per_core, OUT_CH], F32, kind="ExternalOutput")

    rgroups = [list(range(N_CORES))]
    qctr = [0]

    def qn():
        q = qctr[0] % 4
        qctr[0] += 1
        return q

    def allgather(own, sh, half):
        r0 = half * HROWS
        o0 = 1 + half * N_CORES * HROWS
        nc.gpsimd.collective_compute(
            "AllGather", mybir.AluOpType.bypass, replica_groups=rgroups,
            ins=[own[r0:r0 + HROWS, :]], outs=[sh[o0:o0 + N_CORES * HROWS, :]],
        )

    # per-group slot -> tile map
    slot_tile = []
    for gi, (t0, t1) in enumerate(groups):
        st = []
        for t in range(t0, t1):
            st += [t] * chA[t]
        for t in range(t0, t1):
            st += [t] * chB[t]
        slot_tile.append(st)

    with tile.TileContext(nc) as tc:
        with (
            tc.tile_pool(name="const", bufs=1) as cp,
            tc.tile_pool(name="persist", bufs=1) as pp,
            tc.tile_pool(name="xa", bufs=2) as xap,
            tc.tile_pool(name="stage", bufs=3) as sp,
            tc.tile_pool(name="idxp", bufs=3) as ixp,
            tc.tile_pool(name="gath", bufs=3) as gp,
            tc.tile_pool(name="ot", bufs=2) as otp,
            tc.tile_pool(name="oh", bufs=2) as ohp,
            tc.tile_pool(name="rhs", bufs=2) as rp,
            tc.tile_pool(name="small", bufs=3) as smp,
            tc.tile_pool(name="epi", bufs=2) as ep,
            tc.tile_pool(name="psa", bufs=3, space="PSUM") as ppa,
            tc.tile_pool(name="psal", bufs=2, space="PSUM") as pal,
            tc.tile_pool(name="psm", bufs=3, space="PSUM") as ppm,
        ):
            # ---- consts
            W1big = cp.tile([128, 136], F32)
            nc.sync.dma_start(out=W1big[:], in_=W1big_d[:])
            W2big = cp.tile([128, HID + 2], F16)
            nc.sync.dma_start(out=W2big[:], in_=W2big_d[:])
            Wout = cp.tile([HID, OUT_CH], F16)
            nc.sync.dma_start(out=Wout[:], in_=Wout_d[:])
            b1r = cp.tile([128, 128], F32)
            nc.sync.dma_start(out=b1r[:], in_=b1_d[:])
            b2r = cp.tile([128, HID + 2], F32)
            nc.sync.dma_start(out=b2r[:], in_=b2_d[:])
            boutr = cp.tile([128, OUT_CH], F32)
            nc.sync.dma_start(out=boutr[:], in_=bout_d[:])
            ident = cp.tile([128, 128], F16)
            nc.sync.dma_start(out=ident[:], in_=ident_d[:])
            iota = cp.tile([128, 128], F16)
            nc.sync.dma_start(out=iota[:], in_=iota_d[:])
            codes = cp.tile([128, C], F16)
            nc.sync.dma_start(out=codes[:], in_=codes_d[:])

            h_own = pp.tile([128, NT * 128], F16)
            alT1 = pp.tile([128, NT * 8], F16)
            z_own = pp.tile([128, NT * HID], F16)
            alT2 = pp.tile([128, NT * 2], F16)
            wself1 = pp.tile([128, NT, HEADS], F16)
            wself2 = pp.tile([128, NT, 1], F16)

            zs1 = cp.tile([1, T1_COLS], F16)
            nc.vector.memset(zs1[:], 0.0)
            nc.sync.dma_start(out=T1_sh[0:1, :], in_=zs1[:])
            nc.sync.dma_start(out=T1_sh[nrows - 1:nrows, :], in_=zs1[:])
            nc.sync.dma_start(out=T2_sh[0:1, :], in_=zs1[:, 0:T2_COLS])
            nc.sync.dma_start(out=T2_sh[nrows - 1:nrows, :], in_=zs1[:, 0:T2_COLS])

            # ---- phase A
            for t in range(NT):
                xa = xap.tile([128, 128], F32)
                nc.sync.dma_start(out=xa[:], in_=x_slice[:, t * 128:(t + 1) * 128])
                ps = ppa.tile([128, 136], F32, tag="agg")
                nc.tensor.matmul(out=ps[:], lhsT=xa[:], rhs=W1big[:], start=True, stop=True)
                hb = sp.tile([128, T1_COLS], F16, tag="hb")
                nc.vector.tensor_copy(out=hb[:, 0:132], in_=ps[:, 0:132])
                nc.sync.dma_start(out=T1_own[t * 128:(t + 1) * 128, :], in_=hb[:])
                nc.vector.tensor_copy(out=h_own[:, t * 128:(t + 1) * 128], in_=ps[:, 0:128])
                nc.vector.tensor_copy(out=alT1[:, t * 8:t * 8 + 8], in_=ps[:, 128:136])
                if t == NT // 2 - 1:
                    allgather(T1_own, T1_sh, 0)
            allgather(T1_own, T1_sh, 1)

            alT1v = alT1[:].rearrange("p (t e) -> p t e", t=NT)
            xls = smp.tile([128, NT, HEADS], F16, tag="xls")
            nc.vector.tensor_tensor(out=xls[:], in0=alT1v[:, :, 0:4], in1=alT1v[:, :, 4:8],
                                    op=mybir.AluOpType.add)
            e1s = smp.tile([128, NT, HEADS], F16, tag="e1s")
            nc.scalar.activation(e1s[:], xls[:], mybir.ActivationFunctionType.Exp)
            e2s = smp.tile([128, NT, HEADS], F16, tag="e2s")
            nc.scalar.activation(e2s[:], xls[:], mybir.ActivationFunctionType.Exp, scale=NEG_SLOPE)
            nc.vector.tensor_tensor(out=wself1[:], in0=e1s[:], in1=e2s[:], op=mybir.AluOpType.max)

            # ---- layer 1
            for gi, (t0, t1) in enumerate(groups):
                SA, SB = gSA[gi], gSB[gi]
                S = SA + SB
                nt = t1 - t0
                c0 = gc0[gi]
                idxg = ixp.tile([128, S * 8], I16, tag="idx")
                nc.sync.dma_start(out=idxg[:], in_=idx_flat[:, c0 * 8:(c0 + S) * 8])
                G = gp.tile([128, S, T1_COLS], F16, tag="G1")
                if SA:
                    nc.gpsimd.dma_gather(
                        G[:, 0:SA, :], T1_sh[0:b_base, :], idxg[:, 0:SA * 8],
                        128 * SA, 128 * SA, T1_COLS, queue_num=qn(), single_packet=False)
                if SB:
                    nc.gpsimd.dma_gather(
                        G[:, SA:S, :], T1_sh[b_base:nrows, :], idxg[:, SA * 8:S * 8],
                        128 * SB, 128 * SB, T1_COLS, queue_num=qn(), single_packet=False)
                OT = otp.tile([128, S, 128], F16, tag="OT")
                nc.sync.dma_start(out=OT[:], in_=ohT_d[:, c0:c0 + S, :])
                OH = ohp.tile([128, S, 128], F16, tag="OH")
                nc.vector.tensor_tensor(
                    out=OH[:], in0=codes[:, c0:c0 + S, None].to_broadcast([128, S, 128]),
                    in1=iota[:, None, :].to_broadcast([128, S, 128]),
                    op=mybir.AluOpType.is_equal)

                # group-level alD / weights / messages
                alps = pal.tile([128, 4 * S], F32, tag="al")
                for cs_ in range(S):
                    t = slot_tile[gi][cs_]
                    nc.tensor.matmul(out=alps[:, 4 * cs_:4 * cs_ + 4], lhsT=OT[:, cs_, :],
                                     rhs=alT1[:, t * 8 + 4:t * 8 + 8], start=True, stop=True)
                alDs = smp.tile([128, S, HEADS], F16, tag="alDs")
                nc.vector.tensor_copy(out=alDs[:], in_=alps[:].rearrange("p (k e) -> p k e", e=4))
                xl = smp.tile([128, S, HEADS], F16, tag="xl")
                nc.vector.tensor_tensor(out=xl[:], in0=G[:, :, 128:132], in1=alDs[:],
                                        op=mybir.AluOpType.add)
                e1 = smp.tile([128, S, HEADS], F16, tag="e1")
                nc.scalar.activation(e1[:], xl[:], mybir.ActivationFunctionType.Exp)
                e2 = smp.tile([128, S, HEADS], F16, tag="e2")
                nc.scalar.activation(e2[:], xl[:], mybir.ActivationFunctionType.Exp, scale=NEG_SLOPE)
                rhs = rp.tile([128, S + nt, 132], F16, tag="rhs1")
                nc.vector.tensor_tensor(out=rhs[:, 0:S, 128:132], in0=e1[:], in1=e2[:],
                                        op=mybir.AluOpType.max)
                nc.vector.tensor_tensor(
                    out=rhs[:, 0:S, 0:128].rearrange("p k (h j) -> p k h j", h=4),
                    in0=G[:, :, 0:128].rearrange("p k (h j) -> p k h j", h=4),
                    in1=rhs[:, 0:S, 128:132][:, :, :, None].to_broadcast([128, S, 4, 32]),
                    op=mybir.AluOpType.mult)
                nc.vector.tensor_copy(
                    out=rhs[:, S:S + nt, 128:132], in_=wself1[:, t0:t1, :])
                nc.vector.tensor_tensor(
                    out=rhs[:, S:S + nt, 0:128].rearrange("p k (h j) -> p k h j", h=4),
                    in0=h_own[:, t0 * 128:t1 * 128].rearrange("p (t h j) -> p t h j", t=nt, h=4),
                    in1=wself1[:, t0:t1, :, None].to_broadcast([128, nt, 4, 32]),
                    op=mybir.AluOpType.mult)

                psall = ep.tile([128, nt, 132], F32, tag="psall")
                for ti, t in enumerate(range(t0, t1)):
                    ca, cb = chA[t], chB[t]
                    slots = list(range(aoff[t], aoff[t] + ca)) + \
                            list(range(SA + boff[t], SA + boff[t] + cb))
                    ps = ppa.tile([128, 132], F32, tag="agg")
                    for ci, cs_ in enumerate(slots):
                        nc.tensor.matmul(out=ps[:], lhsT=OH[:, cs_, :], rhs=rhs[:, cs_, :],
                                         start=(ci == 0), stop=False)
                    nc.tensor.matmul(out=ps[:], lhsT=ident[:], rhs=rhs[:, S + ti, :],
                                     start=False, stop=True)
                    nc.vector.tensor_copy(out=psall[:, ti, :], in_=ps[:])
                psg_v = psall[:]

                # group-level epilogue
                rec = smp.tile([128, nt, HEADS], F32, tag="rec")
                nc.vector.reciprocal(out=rec[:], in_=psg_v[:, :, 128:132])
                y1 = ep.tile([128, nt, 128], F16, tag="y")
                nc.vector.tensor_tensor(
                    out=y1[:].rearrange("p t (h j) -> p t h j", h=4),
                    in0=psg_v[:, :, 0:128].rearrange("p t (h j) -> p t h j", h=4),
                    in1=rec[:, :, :, None].to_broadcast([128, nt, 4, 32]),
                    op=mybir.AluOpType.mult)
                nc.vector.tensor_tensor(
                    out=y1[:], in0=y1[:],
                    in1=b1r[:, None, :].to_broadcast([128, nt, 128]),
                    op=mybir.AluOpType.add)
                m1 = ep.tile([128, nt, 128], F16, tag="m1")
                nc.vector.tensor_scalar(out=m1[:], in0=y1[:], scalar1=0.0, scalar2=None,
                                        op0=mybir.AluOpType.min)
                nc.scalar.activation(m1[:], m1[:], mybir.ActivationFunctionType.Exp)
                nc.vector.tensor_scalar(out=y1[:], in0=y1[:], scalar1=0.0, scalar2=-1.0,
                                        op0=mybir.AluOpType.max, op1=mybir.AluOpType.add)
                h2 = m1
                nc.vector.tensor_tensor(out=h2[:], in0=m1[:], in1=y1[:], op=mybir.AluOpType.add)

                t2g = sp.tile([128, nt, T2_COLS], F16, tag="t2b")
                for ti, t in enumerate(range(t0, t1)):
                    pt = ppm.tile([128, 128], F16, tag="misc")
                    nc.tensor.transpose(out=pt[:], in_=h2[:, ti, :], identity=ident[:])
                    h2T = ep.tile([128, 128], F16, tag="h2T")
                    nc.vector.tensor_copy(out=h2T[:], in_=pt[:])
                    psz = ppm.tile([128, HID + 2], F32, tag="misc")
                    nc.tensor.matmul(out=psz[:], lhsT=h2T[:], rhs=W2big[:], start=True, stop=True)
                    nc.vector.tensor_tensor(out=t2g[:, ti, 0:HID + 2], in0=psz[:], in1=b2r[:],
                                            op=mybir.AluOpType.add)
                nc.sync.dma_start(
                    out=T2_own[t0 * 128:t1 * 128, :].rearrange("(t p) c -> p t c", p=128),
                    in_=t2g[:])
                nc.vector.tensor_copy(
                    out=z_own[:, t0 * HID:t1 * HID].rearrange("p (t c) -> p t c", t=nt),
                    in_=t2g[:, :, 0:HID])
                nc.vector.tensor_copy(
                    out=alT2[:, t0 * 2:t1 * 2].rearrange("p (t c) -> p t c", t=nt),
                    in_=t2g[:, :, HID:HID + 2])
                if t0 < NT // 2 <= t1:
                    allgather(T2_own, T2_sh, 0)
            allgather(T2_own, T2_sh, 1)

            alT2v = alT2[:].rearrange("p (t e) -> p t e", t=NT)
            xls2 = smp.tile([128, NT, 1], F16, tag="xls2")
            nc.vector.tensor_tensor(out=xls2[:], in0=alT2v[:, :, 0:1], in1=alT2v[:, :, 1:2],
                                    op=mybir.AluOpType.add)
            e1s2 = smp.tile([128, NT, 1], F16, tag="e1s2")
            nc.scalar.activation(e1s2[:], xls2[:], mybir.ActivationFunctionType.Exp)
            e2s2 = smp.tile([128, NT, 1], F16, tag="e2s2")
            nc.scalar.activation(e2s2[:], xls2[:], mybir.ActivationFunctionType.Exp, scale=NEG_SLOPE)
            nc.vector.tensor_tensor(out=wself2[:], in0=e1s2[:], in1=e2s2[:], op=mybir.AluOpType.max)

            # ---- layer 2
            for gi, (t0, t1) in enumerate(groups):
                SA, SB = gSA[gi], gSB[gi]
                S = SA + SB
                nt = t1 - t0
                c0 = gc0[gi]
                idxg = ixp.tile([128, S * 8], I16, tag="idx")
                nc.sync.dma_start(out=idxg[:], in_=idx_flat[:, c0 * 8:(c0 + S) * 8])
                G2 = gp.tile([128, S, T2_COLS], F16, tag="G2")
                if SA:
                    nc.gpsimd.dma_gather(
                        G2[:, 0:SA, :], T2_sh[0:b_base, :], idxg[:, 0:SA * 8],
                        128 * SA, 128 * SA, T2_COLS, queue_num=qn(), single_packet=False)
                if SB:
                    nc.gpsimd.dma_gather(
                        G2[:, SA:S, :], T2_sh[b_base:nrows, :], idxg[:, SA * 8:S * 8],
                        128 * SB, 128 * SB, T2_COLS, queue_num=qn(), single_packet=False)
                OT = otp.tile([128, S, 128], F16, tag="OT")
                nc.sync.dma_start(out=OT[:], in_=ohT_d[:, c0:c0 + S, :])
                OH = ohp.tile([128, S, 128], F16, tag="OH")
                nc.vector.tensor_tensor(
                    out=OH[:], in0=codes[:, c0:c0 + S, None].to_broadcast([128, S, 128]),
                    in1=iota[:, None, :].to_broadcast([128, S, 128]),
                    op=mybir.AluOpType.is_equal)

                alps2 = pal.tile([128, S], F32, tag="al")
                for cs_ in range(S):
                    t = slot_tile[gi][cs_]
                    nc.tensor.matmul(out=alps2[:, cs_:cs_ + 1], lhsT=OT[:, cs_, :],
                                     rhs=alT2[:, t * 2 + 1:t * 2 + 2], start=True, stop=True)
                alDs2 = smp.tile([128, S, 1], F16, tag="alDs2")
                nc.vector.tensor_copy(out=alDs2[:], in_=alps2[:, :, None])
                xl2 = smp.tile([128, S, 1], F16, tag="xl2")
                nc.vector.tensor_tensor(out=xl2[:], in0=G2[:, :, 32:33], in1=alDs2[:],
                                        op=mybir.AluOpType.add)
                e1b = smp.tile([128, S, 1], F16, tag="e1b")
                nc.scalar.activation(e1b[:], xl2[:], mybir.ActivationFunctionType.Exp)
                e2b = smp.tile([128, S, 1], F16, tag="e2b")
                nc.scalar.activation(e2b[:], xl2[:], mybir.ActivationFunctionType.Exp, scale=NEG_SLOPE)
                rhs2 = rp.tile([128, S + nt, HID + 1], F16, tag="rhs2")
                nc.vector.tensor_tensor(out=rhs2[:, 0:S, HID:HID + 1], in0=e1b[:], in1=e2b[:],
                                        op=mybir.AluOpType.max)
                nc.vector.tensor_tensor(
                    out=rhs2[:, 0:S, 0:HID], in0=G2[:, :, 0:HID],
                    in1=rhs2[:, 0:S, HID:HID + 1].to_broadcast([128, S, HID]),
                    op=mybir.AluOpType.mult)
                nc.vector.tensor_copy(
                    out=rhs2[:, S:S + nt, HID:HID + 1], in_=wself2[:, t0:t1, :])
                nc.vector.tensor_tensor(
                    out=rhs2[:, S:S + nt, 0:HID],
                    in0=z_own[:, t0 * HID:t1 * HID].rearrange("p (t c) -> p t c", t=nt),
                    in1=wself2[:, t0:t1, :].to_broadcast([128, nt, HID]),
                    op=mybir.AluOpType.mult)

                psall2 = ep.tile([128, nt, 33], F32, tag="psall2")
                for ti, t in enumerate(range(t0, t1)):
                    ca, cb = chA[t], chB[t]
                    slots = list(range(aoff[t], aoff[t] + ca)) + \
                            list(range(SA + boff[t], SA + boff[t] + cb))
                    ps2 = ppa.tile([128, 33], F32, tag="agg")
                    for ci, cs_ in enumerate(slots):
                        nc.tensor.matmul(out=ps2[:], lhsT=OH[:, cs_, :], rhs=rhs2[:, cs_, :],
                                         start=(ci == 0), stop=False)
                    nc.tensor.matmul(out=ps2[:], lhsT=ident[:], rhs=rhs2[:, S + ti, :],
                                     start=False, stop=True)
                    nc.vector.tensor_copy(out=psall2[:, ti, :], in_=ps2[:])
                psg2_v = psall2[:]

                rec2 = smp.tile([128, nt, 1], F32, tag="rec2")
                nc.vector.reciprocal(out=rec2[:], in_=psg2_v[:, :, HID:HID + 1])
                y2 = ep.tile([128, nt, HID], F16, tag="y2")
                nc.vector.tensor_tensor(out=y2[:], in0=psg2_v[:, :, 0:HID],
                                        in1=rec2[:].to_broadcast([128, nt, HID]),
                                        op=mybir.AluOpType.mult)
                m2 = ep.tile([128, nt, HID], F16, tag="m2")
                nc.vector.tensor_scalar(out=m2[:], in0=y2[:], scalar1=0.0, scalar2=None,
                                        op0=mybir.AluOpType.min)
                nc.scalar.activation(m2[:], m2[:], mybir.ActivationFunctionType.Exp)
                nc.vector.tensor_scalar(out=y2[:], in0=y2[:], scalar1=0.0, scalar2=-1.0,
                                        op0=mybir.AluOpType.max, op1=mybir.AluOpType.add)
                h3 = m2
                nc.vector.tensor_tensor(out=h3[:], in0=m2[:], in1=y2[:], op=mybir.AluOpType.add)

                outg = ep.tile([128, nt, OUT_CH], F32, tag="outf")
                for ti, t in enumerate(range(t0, t1)):
                    pt2 = ppm.tile([128, 128], F16, tag="misc")
                    nc.tensor.transpose(out=pt2[:HID, :], in_=h3[:, ti, :], identity=ident[:])
                    h3T = ep.tile([HID, 128], F16, tag="h3T")
                    nc.vector.tensor_copy(out=h3T[:], in_=pt2[:HID, :])
                    psf = ppm.tile([128, OUT_CH], F32, tag="misc")
                    nc.tensor.matmul(out=psf[:], lhsT=h3T[:], rhs=Wout[:], start=True, stop=True)
                    nc.vector.tensor_tensor(out=outg[:, ti, :], in0=psf[:], in1=boutr[:],
                                            op=mybir.AluOpType.add)
                nc.sync.dma_start(
                    out=out_d[t0 * 128:t1 * 128, :].rearrange("(t p) c -> p t c", p=128),
                    in_=outg[:])

    nc.compile()
    return nc


def _run(inputs, trace=False):
    meta, in_maps = _prep(**inputs)
    nc = _build(meta)
    res = run_bass_kernel_spmd(nc, in_maps, core_ids=list(range(N_CORES)), trace=trace)
    outg = np.concatenate([res.results[c]["out"] for c in range(N_CORES)], axis=0)
    out_nodes = np.empty((meta["n_pad"], OUT_CH), np.float32)
    out_nodes[meta["perm_rows"]] = outg
    return out_nodes[:meta["N"]], res


def kernel(**inputs):
    out, _ = _run(inputs, trace=False)
    return out


# revision 14
# speedup vs baseline: 2.5294x; 1.0649x over previous
"""GAT (2-layer, 4-head then 1-head) on 8 Trainium2 NeuronCores.

Strategy (v3 — dense one-hot chunks, group-level batching)
----------------------------------------------------------
- Nodes degree-sorted and dealt round-robin to 8 cores; each core's 5120
  nodes form 40 dst tiles of 128.
- Edges (self-loops excluded) are packed DENSELY per (tile, table-half)
  into 128-edge chunks (pad ~9%). Aggregation multiplies each chunk by a
  one-hot lhsT whose column d selects the partitions holding edges of
  dst d, accumulating Sum_e w_e*[h_e | 1] per dst in PSUM.
- Per-edge al_dst: alD[p] = onehotT_c (contract dst) al8_own — computed
  for ALL chunks of a group into one PSUM bank, one matmul per chunk.
  onehotT is streamed from DRAM; the aggregation one-hot is generated on
  device (DVE is_equal(codes, iota)).
- All per-edge DVE work (al add, exp, max, message multiply) happens at
  GROUP granularity (~36 chunks per instruction), not per tile; only the
  aggregation matmuls and the epilogue are per tile.
- Self-loops never gathered: extra rhs slots + identity-matmul chunks
  fed from SBUF-resident h_own / z_own.
- Softmax without max-subtraction (exp(lrelu(x)) = max(exp x, exp .2x));
  denominators ride the same one-hot matmul (w columns). Pad slots have
  all-zero one-hot columns.
- Node tables: L1 rows [h(128)|al_src(4)|pad] 512B; L2 rows
  [z+b2(32)|as2(1)|pad] 256B. Table rows are numbered CHUNK-MAJOR
  (pos<2560 first for all cores, then the rest) so each half-table
  AllGather has a contiguous output and can overlap compute. int16
  gather indices via the A/B table split at row 30721.
"""

import numpy as np

import concourse.bacc as bacc
import concourse.mybir as mybir
import concourse.tile as tile
from concourse.bass_utils import run_bass_kernel_spmd

F32 = mybir.dt.float32
F16 = mybir.dt.float16
I16 = mybir.dt.int16

IN_CH = 128
HID = 32
HEADS = 4
OUT_CH = 112
NEG_SLOPE = 0.2

T1_COLS = 256
T2_COLS = 128

N_CORES = 8
GCAP = 36
PAD_CODE = 200
HROWS = 2560  # rows per core per allgather chunk (2 chunks)


def _prep(x, edge_index, W1, a_src1, a_dst1, b1, W2, a_src2, a_dst2, b2, W_out, b_out):
    N = x.shape[0]
    per_core = -(-N // (N_CORES * 128)) * 128
    n_pad = per_core * N_CORES
    NT = per_core // 128
    nrows = n_pad + 2
    b_base = 1 + N_CORES * HROWS
    assert b_base - 1 <= 32767 and nrows - b_base <= 32767
    assert per_core == 2 * HROWS

    src = np.asarray(edge_index[0], np.int64)
    dst = np.asarray(edge_index[1], np.int64)
    E = src.shape[0]

    deg = np.bincount(dst, minlength=n_pad)
    order = np.argsort(deg, kind="stable")
    rank = np.empty(n_pad, np.int64)
    rank[order] = np.arange(n_pad)
    coreid = rank % N_CORES
    pos = rank // N_CORES
    grow = coreid * per_core + pos
    # chunk-major table rows: all cores' pos<HROWS first, then the rest
    trow = 1 + (pos // HROWS) * (N_CORES * HROWS) + coreid * HROWS + pos % HROWS
    perm_rows = np.empty(n_pad, np.int64)
    perm_rows[grow] = np.arange(n_pad)

    sr = trow[src]
    dr = grow[dst]
    gB = sr >= b_base
    core = dr // per_core
    tl = (dr % per_core) // 128
    lane = dr % 128

    EaT = np.zeros((N_CORES, NT), np.int64)
    EbT = np.zeros((N_CORES, NT), np.int64)
    np.add.at(EaT, (core[~gB], tl[~gB]), 1)
    np.add.at(EbT, (core[gB], tl[gB]), 1)
    chA = (-(-EaT // 128)).max(axis=0)
    chB = (-(-EbT // 128)).max(axis=0)

    groups = []
    t = 0
    while t < NT:
        e = t
        tot = 0
        while e < NT and (e == t or tot + chA[e] + chB[e] <= GCAP):
            tot += chA[e] + chB[e]
            e += 1
        groups.append((t, e))
        t = e

    aoff = np.zeros(NT, np.int64)
    boff = np.zeros(NT, np.int64)
    gc0 = []
    gSA = []
    gSB = []
    C = 0
    g_of_tile = np.zeros(NT, np.int64)
    for gi, (t0, t1) in enumerate(groups):
        sa = int(chA[t0:t1].sum())
        sb = int(chB[t0:t1].sum())
        gc0.append(C)
        gSA.append(sa)
        gSB.append(sb)
        off = 0
        for t in range(t0, t1):
            g_of_tile[t] = gi
            aoff[t] = off
            off += chA[t]
        off = 0
        for t in range(t0, t1):
            boff[t] = off
            off += chB[t]
        C += sa + sb
    totidx = C * 128
    assert totidx % 16 == 0

    SENT_A = 0
    SENT_B = nrows - 1 - b_base
    idx_streams = np.zeros((N_CORES, C, 128), np.int16)
    for gi in range(len(groups)):
        idx_streams[:, gc0[gi]:gc0[gi] + gSA[gi], :] = SENT_A
        idx_streams[:, gc0[gi] + gSA[gi]:gc0[gi] + gSA[gi] + gSB[gi], :] = SENT_B
    codes_streams = np.full((N_CORES, C, 128), PAD_CODE, np.int16)

    cbaseA = np.array([gc0[g_of_tile[t]] + aoff[t] for t in range(NT)])
    cbaseB = np.array([gc0[g_of_tile[t]] + gSA[g_of_tile[t]] + boff[t] for t in range(NT)])

    key = (core * NT + tl) * 2 + gB.astype(np.int64)
    eorder = np.argsort(key, kind="stable")
    ks = key[eorder]
    newrun = np.ones(E, bool)
    newrun[1:] = ks[1:] != ks[:-1]
    run_start = np.flatnonzero(newrun)
    run_id = np.cumsum(newrun) - 1
    j = np.arange(E) - run_start[run_id]
    cs = core[eorder]
    tls = tl[eorder]
    gs = gB[eorder]
    cidx = np.where(gs, cbaseB[tls], cbaseA[tls]) + j // 128
    idx_streams[cs, cidx, j % 128] = np.where(gs, sr[eorder] - b_base, sr[eorder]).astype(np.int16)
    codes_streams[cs, cidx, j % 128] = lane[eorder]

    idx_wrapped = np.empty((N_CORES, 128, totidx // 16), np.int16)
    for c in range(N_CORES):
        w16 = idx_streams[c].reshape(-1, 16).T
        idx_wrapped[c] = np.tile(w16, (8, 1))

    f16 = np.float16
    codes_pc = np.transpose(codes_streams, (0, 2, 1)).astype(f16)
    d_ar = np.arange(128, dtype=np.int16)[:, None, None]
    ohT = np.empty((N_CORES, 128, C, 128), f16)
    for c in range(N_CORES):
        ohT[c] = (codes_streams[c][None, :, :] == d_ar).astype(f16)

    xp = np.zeros((n_pad, IN_CH), np.float32)
    xp[:N] = np.asarray(x, np.float32)
    x_slices = np.empty((N_CORES, IN_CH, per_core), np.float32)
    for c in range(N_CORES):
        x_slices[c] = xp[perm_rows[c * per_core:(c + 1) * per_core]].T

    W1 = np.asarray(W1, np.float32)
    Bsrc = np.zeros((HEADS * HID, HEADS), np.float32)
    Bdst = np.zeros((HEADS * HID, HEADS), np.float32)
    for h in range(HEADS):
        Bsrc[h * HID:(h + 1) * HID, h] = np.asarray(a_src1[h], np.float32)
        Bdst[h * HID:(h + 1) * HID, h] = np.asarray(a_dst1[h], np.float32)
    W1big = np.concatenate([W1, W1 @ Bsrc, W1 @ Bdst], axis=1)
    W2 = np.asarray(W2, np.float32)
    W2big = np.concatenate(
        [W2, W2 @ np.asarray(a_src2, np.float32).T, W2 @ np.asarray(a_dst2, np.float32).T],
        axis=1,
    )
    b1_rep = np.tile(np.asarray(b1, np.float32)[None, :], (128, 1))
    b2_rep = np.zeros((128, HID + 2), np.float32)
    b2_rep[:, :HID] = np.asarray(b2, np.float32)[None, :]
    bout_rep = np.tile(np.asarray(b_out, np.float32)[None, :], (128, 1))
    ident = np.eye(128, dtype=f16)
    iota = np.tile(np.arange(128, dtype=f16)[None, :], (128, 1))

    meta = dict(
        N=N, n_pad=n_pad, per_core=per_core, NT=NT, nrows=nrows, b_base=b_base,
        chA=chA.tolist(), chB=chB.tolist(), groups=groups, gc0=gc0, gSA=gSA,
        gSB=gSB, aoff=aoff.tolist(), boff=boff.tolist(), C=C, totidx=totidx,
        perm_rows=perm_rows,
    )
    shared = dict(
        W1big=W1big, W2big=W2big.astype(f16), Wout=np.asarray(W_out, np.float32).astype(f16),
        b1_rep=b1_rep, b2_rep=b2_rep, bout_rep=bout_rep, ident=ident, iota=iota,
    )
    in_maps = []
    for c in range(N_CORES):
        m = dict(shared)
        m["x_slice"] = np.ascontiguousarray(x_slices[c])
        m["idx_flat"] = np.ascontiguousarray(idx_wrapped[c])
        m["codes"] = np.ascontiguousarray(codes_pc[c])
        m["ohT"] = np.ascontiguousarray(ohT[c])
        in_maps.append(m)
    return meta, in_maps


def _build(meta):
    per_core, NT, nrows, b_base = meta["per_core"], meta["NT"], meta["nrows"], meta["b_base"]
    chA, chB = meta["chA"], meta["chB"]
    groups, gc0, gSA, gSB = meta["groups"], meta["gc0"], meta["gSA"], meta["gSB"]
    aoff, boff, C = meta["aoff"], meta["boff"], meta["C"]
    totidx = meta["totidx"]

    nc = bacc.Bacc("TRN2", num_devices=N_CORES, num_swdge_queues=4,
                   dynamic_dma_scratch_size=32768)

    x_slice = nc.dram_tensor("x_slice", [IN_CH, per_core], F32, kind="ExternalInput")
    idx_flat = nc.dram_tensor("idx_flat", [128, totidx // 16], I16, kind="ExternalInput")
    codes_d = nc.dram_tensor("codes", [128, C], F16, kind="ExternalInput")
    ohT_d = nc.dram_tensor("ohT", [128, C, 128], F16, kind="ExternalInput")
    W1big_d = nc.dram_tensor("W1big", [128, 136], F32, kind="ExternalInput")
    W2big_d = nc.dram_tensor("W2big", [128, HID + 2], F16, kind="ExternalInput")
    Wout_d = nc.dram_tensor("Wout", [HID, OUT_CH], F16, kind="ExternalInput")
    b1_d = nc.dram_tensor("b1_rep", [128, 128], F32, kind="ExternalInput")
    b2_d = nc.dram_tensor("b2_rep", [128, HID + 2], F32, kind="ExternalInput")
    bout_d = nc.dram_tensor("bout_rep", [128, OUT_CH], F32, kind="ExternalInput")
    ident_d = nc.dram_tensor("ident", [128, 128], F16, kind="ExternalInput")
    iota_d = nc.dram_tensor("iota", [128, 128], F16, kind="ExternalInput")

    T1_own = nc.dram_tensor("T1_own", [per_core, T1_COLS], F16, kind="Internal")
    T1_sh = nc.dram_tensor("T1_sh", [nrows, T1_COLS], F16, kind="Internal", addr_space="Shared")
    T2_own = nc.dram_tensor("T2_own", [per_core, T2_COLS], F16, kind="Internal")
    T2_sh = nc.dram_tensor("T2_sh", [nrows, T2_COLS], F16, kind="Internal", addr_space="Shared")
    out_d = nc.dram_tensor("out", [per_core, OUT_CH], F32, kind="ExternalOutput")

    rgroups = [list(range(N_CORES))]
    qctr = [0]

    def qn():
        q = qctr[0] % 4
        qctr[0] += 1
        return q

    def allgather(own, sh, half):
        r0 = half * HROWS
        o0 = 1 + half * N_CORES * HROWS
        nc.gpsimd.collective_compute(
            "AllGather", mybir.AluOpType.bypass, replica_groups=rgroups,
            ins=[own[r0:r0 + HROWS, :]], outs=[sh[o0:o0 + N_CORES * HROWS, :]],
        )

    # per-group slot -> tile map
    slot_tile = []
    for gi, (t0, t1) in enumerate(groups):
        st = []
        for t in range(t0, t1):
            st += [t] * chA[t]
        for t in range(t0, t1):
            st += [t] * chB[t]
        slot_tile.append(st)

    with tile.TileContext(nc) as tc:
        with (
            tc.tile_pool(name="const", bufs=1) as cp,
            tc.tile_pool(name="persist", bufs=1) as pp,
            tc.tile_pool(name="xa", bufs=2) as xap,
            tc.tile_pool(name="stage", bufs=3) as sp,
            tc.tile_pool(name="idxp", bufs=3) as ixp,
            tc.tile_pool(name="gath", bufs=3) as gp,
            tc.tile_pool(name="ot", bufs=2) as otp,
            tc.tile_pool(name="oh", bufs=2) as ohp,
            tc.tile_pool(name="rhs", bufs=2) as rp,
            tc.tile_pool(name="small", bufs=3) as smp,
            tc.tile_pool(name="epi", bufs=2) as ep,
            tc.tile_pool(name="psa", bufs=3, space="PSUM") as ppa,
            tc.tile_pool(name="psal", bufs=2, space="PSUM") as pal,
            tc.tile_pool(name="psm", bufs=3, space="PSUM") as ppm,
        ):
            # ---- consts
            W1big = cp.tile([128, 136], F32)
            nc.sync.dma_start(out=W1big[:], in_=W1big_d[:])
            W2big = cp.tile([128, HID + 2], F16)
            nc.sync.dma_start(out=W2big[:], in_=W2big_d[:])
            Wout = cp.tile([HID, OUT_CH], F16)
            nc.sync.dma_start(out=Wout[:], in_=Wout_d[:])
            b1r = cp.tile([128, 128], F32)
            nc.sync.dma_start(out=b1r[:], in_=b1_d[:])
            b2r = cp.tile([128, HID + 2], F32)
            nc.sync.dma_start(out=b2r[:], in_=b2_d[:])
            boutr = cp.tile([128, OUT_CH], F32)
            nc.sync.dma_start(out=boutr[:], in_=bout_d[:])
            ident = cp.tile([128, 128], F16)
            nc.sync.dma_start(out=ident[:], in_=ident_d[:])
            iota = cp.tile([128, 128], F16)
            nc.sync.dma_start(out=iota[:], in_=iota_d[:])
            codes = cp.tile([128, C], F16)
            nc.sync.dma_start(out=codes[:], in_=codes_d[:])

            h_own = pp.tile([128, NT * 128], F16)
            alT1 = pp.tile([128, NT * 8], F16)
            z_own = pp.tile([128, NT * HID], F16)
            alT2 = pp.tile([128, NT * 2], F16)
            wself1 = pp.tile([128, NT, HEADS], F16)
            wself2 = pp.tile([128, NT, 1], F16)

            zs1 = cp.tile([1, T1_COLS], F16)
            nc.vector.memset(zs1[:], 0.0)
            nc.sync.dma_start(out=T1_sh[0:1, :], in_=zs1[:])
            nc.sync.dma_start(out=T1_sh[nrows - 1:nrows, :], in_=zs1[:])
            nc.sync.dma_start(out=T2_sh[0:1, :], in_=zs1[:, 0:T2_COLS])
            nc.sync.dma_start(out=T2_sh[nrows - 1:nrows, :], in_=zs1[:, 0:T2_COLS])

            # ---- phase A
            for t in range(NT):
                xa = xap.tile([128, 128], F32)
                nc.sync.dma_start(out=xa[:], in_=x_slice[:, t * 128:(t + 1) * 128])
                ps = ppa.tile([128, 136], F32, tag="agg")
                nc.tensor.matmul(out=ps[:], lhsT=xa[:], rhs=W1big[:], start=True, stop=True)
                hb = sp.tile([128, T1_COLS], F16, tag="hb")
                nc.vector.tensor_copy(out=hb[:, 0:132], in_=ps[:, 0:132])
                nc.scalar.dma_start(out=T1_own[t * 128:(t + 1) * 128, :], in_=hb[:])
                nc.vector.tensor_copy(out=h_own[:, t * 128:(t + 1) * 128], in_=ps[:, 0:128])
                nc.vector.tensor_copy(out=alT1[:, t * 8:t * 8 + 8], in_=ps[:, 128:136])
                if t == NT // 2 - 1:
                    allgather(T1_own, T1_sh, 0)
            allgather(T1_own, T1_sh, 1)

            alT1v = alT1[:].rearrange("p (t e) -> p t e", t=NT)
            xls = smp.tile([128, NT, HEADS], F16, tag="xls")
            nc.vector.tensor_tensor(out=xls[:], in0=alT1v[:, :, 0:4], in1=alT1v[:, :, 4:8],
                                    op=mybir.AluOpType.add)
            e1s = smp.tile([128, NT, HEADS], F16, tag="e1s")
            nc.scalar.activation(e1s[:], xls[:], mybir.ActivationFunctionType.Exp)
            e2s = smp.tile([128, NT, HEADS], F16, tag="e2s")
            nc.scalar.activation(e2s[:], xls[:], mybir.ActivationFunctionType.Exp, scale=NEG_SLOPE)
            nc.vector.tensor_tensor(out=wself1[:], in0=e1s[:], in1=e2s[:], op=mybir.AluOpType.max)

            # ---- layer 1
            for gi, (t0, t1) in enumerate(groups):
                SA, SB = gSA[gi], gSB[gi]
                S = SA + SB
                nt = t1 - t0
                c0 = gc0[gi]
                idxg = ixp.tile([128, S * 8], I16, tag="idx")
                nc.sync.dma_start(out=idxg[:], in_=idx_flat[:, c0 * 8:(c0 + S) * 8])
                G = gp.tile([128, S, T1_COLS], F16, tag="G1")
                if SA:
                    nc.gpsimd.dma_gather(
                        G[:, 0:SA, :], T1_sh[0:b_base, :], idxg[:, 0:SA * 8],
                        128 * SA, 128 * SA, T1_COLS, queue_num=qn(), single_packet=False)
                if SB:
                    nc.gpsimd.dma_gather(
                        G[:, SA:S, :], T1_sh[b_base:nrows, :], idxg[:, SA * 8:S * 8],
                        128 * SB, 128 * SB, T1_COLS, queue_num=qn(), single_packet=False)
                OT = otp.tile([128, S, 128], F16, tag="OT")
                nc.scalar.dma_start(out=OT[:], in_=ohT_d[:, c0:c0 + S, :])
                OH = ohp.tile([128, S, 128], F16, tag="OH")
                nc.vector.tensor_tensor(
                    out=OH[:], in0=codes[:, c0:c0 + S, None].to_broadcast([128, S, 128]),
                    in1=iota[:, None, :].to_broadcast([128, S, 128]),
                    op=mybir.AluOpType.is_equal)

                # group-level alD / weights / messages
                alps = pal.tile([128, 4 * S], F32, tag="al")
                for cs_ in range(S):
                    t = slot_tile[gi][cs_]
                    nc.tensor.matmul(out=alps[:, 4 * cs_:4 * cs_ + 4], lhsT=OT[:, cs_, :],
                                     rhs=alT1[:, t * 8 + 4:t * 8 + 8], start=True, stop=True)
                alDs = smp.tile([128, S, HEADS], F16, tag="alDs")
                nc.vector.tensor_copy(out=alDs[:], in_=alps[:].rearrange("p (k e) -> p k e", e=4))
                xl = smp.tile([128, S, HEADS], F16, tag="xl")
                nc.vector.tensor_tensor(out=xl[:], in0=G[:, :, 128:132], in1=alDs[:],
                                        op=mybir.AluOpType.add)
                e1 = smp.tile([128, S, HEADS], F16, tag="e1")
                nc.scalar.activation(e1[:], xl[:], mybir.ActivationFunctionType.Exp)
                e2 = smp.tile([128, S, HEADS], F16, tag="e2")
                nc.scalar.activation(e2[:], xl[:], mybir.ActivationFunctionType.Exp, scale=NEG_SLOPE)
                rhs = rp.tile([128, S + nt, 132], F16, tag="rhs1")
                nc.vector.tensor_tensor(out=rhs[:, 0:S, 128:132], in0=e1[:], in1=e2[:],
                                        op=mybir.AluOpType.max)
                nc.vector.tensor_tensor(
                    out=rhs[:, 0:S, 0:128].rearrange("p k (h j) -> p k h j", h=4),
                    in0=G[:, :, 0:128].rearrange("p k (h j) -> p k h j", h=4),
                    in1=rhs[:, 0:S, 128:132][:, :, :, None].to_broadcast([128, S, 4, 32]),
                    op=mybir.AluOpType.mult)
                nc.vector.tensor_copy(
                    out=rhs[:, S:S + nt, 128:132], in_=wself1[:, t0:t1, :])
                nc.vector.tensor_tensor(
                    out=rhs[:, S:S + nt, 0:128].rearrange("p k (h j) -> p k h j", h=4),
                    in0=h_own[:, t0 * 128:t1 * 128].rearrange("p (t h j) -> p t h j", t=nt, h=4),
                    in1=wself1[:, t0:t1, :, None].to_broadcast([128, nt, 4, 32]),
                    op=mybir.AluOpType.mult)

                psall = ep.tile([128, nt, 132], F32, tag="psall")
                for ti, t in enumerate(range(t0, t1)):
                    ca, cb = chA[t], chB[t]
                    slots = list(range(aoff[t], aoff[t] + ca)) + \
                            list(range(SA + boff[t], SA + boff[t] + cb))
                    ps = ppa.tile([128, 132], F32, tag="agg")
                    for ci, cs_ in enumerate(slots):
                        nc.tensor.matmul(out=ps[:], lhsT=OH[:, cs_, :], rhs=rhs[:, cs_, :],
                                         start=(ci == 0), stop=False)
                    nc.tensor.matmul(out=ps[:], lhsT=ident[:], rhs=rhs[:, S + ti, :],
                                     start=False, stop=True)
                    nc.vector.tensor_copy(out=psall[:, ti, :], in_=ps[:])
                psg_v = psall[:]

                # group-level epilogue
                rec = smp.tile([128, nt, HEADS], F32, tag="rec")
                nc.vector.reciprocal(out=rec[:], in_=psg_v[:, :, 128:132])
                y1 = ep.tile([128, nt, 128], F16, tag="y")
                nc.vector.tensor_tensor(
                    out=y1[:].rearrange("p t (h j) -> p t h j", h=4),
                    in0=psg_v[:, :, 0:128].rearrange("p t (h j) -> p t h j", h=4),
                    in1=rec[:, :, :, None].to_broadcast([128, nt, 4, 32]),
                    op=mybir.AluOpType.mult)
                nc.vector.tensor_tensor(
                    out=y1[:], in0=y1[:],
                    in1=b1r[:, None, :].to_broadcast([128, nt, 128]),
                    op=mybir.AluOpType.add)
                m1 = ep.tile([128, nt, 128], F16, tag="m1")
                nc.vector.tensor_scalar(out=m1[:], in0=y1[:], scalar1=0.0, scalar2=None,
                                        op0=mybir.AluOpType.min)
                nc.scalar.activation(m1[:], m1[:], mybir.ActivationFunctionType.Exp)
                nc.vector.tensor_scalar(out=y1[:], in0=y1[:], scalar1=0.0, scalar2=-1.0,
                                        op0=mybir.AluOpType.max, op1=mybir.AluOpType.add)
                h2 = m1
                nc.vector.tensor_tensor(out=h2[:], in0=m1[:], in1=y1[:], op=mybir.AluOpType.add)

                t2g = sp.tile([128, nt, T2_COLS], F16, tag="t2b")
                for ti, t in enumerate(range(t0, t1)):
                    pt = ppm.tile([128, 128], F16, tag="misc")
                    nc.tensor.transpose(out=pt[:], in_=h2[:, ti, :], identity=ident[:])
                    h2T = ep.tile([128, 128], F16, tag="h2T")
                    nc.vector.tensor_copy(out=h2T[:], in_=pt[:])
                    psz = ppm.tile([128, HID + 2], F32, tag="misc")
                    nc.tensor.matmul(out=psz[:], lhsT=h2T[:], rhs=W2big[:], start=True, stop=True)
                    nc.vector.tensor_tensor(out=t2g[:, ti, 0:HID + 2], in0=psz[:], in1=b2r[:],
                                            op=mybir.AluOpType.add)
                nc.scalar.dma_start(
                    out=T2_own[t0 * 128:t1 * 128, :].rearrange("(t p) c -> p t c", p=128),
                    in_=t2g[:])
                nc.vector.tensor_copy(
                    out=z_own[:, t0 * HID:t1 * HID].rearrange("p (t c) -> p t c", t=nt),
                    in_=t2g[:, :, 0:HID])
                nc.vector.tensor_copy(
                    out=alT2[:, t0 * 2:t1 * 2].rearrange("p (t c) -> p t c", t=nt),
                    in_=t2g[:, :, HID:HID + 2])
                if t0 < NT // 2 <= t1:
                    allgather(T2_own, T2_sh, 0)
            allgather(T2_own, T2_sh, 1)

            alT2v = alT2[:].rearrange("p (t e) -> p t e", t=NT)
            xls2 = smp.tile([128, NT, 1], F16, tag="xls2")
            nc.vector.tensor_tensor(out=xls2[:], in0=alT2v[:, :, 0:1], in1=alT2v[:, :, 1:2],
                                    op=mybir.AluOpType.add)
            e1s2 = smp.tile([128, NT, 1], F16, tag="e1s2")
            nc.scalar.activation(e1s2[:], xls2[:], mybir.ActivationFunctionType.Exp)
            e2s2 = smp.tile([128, NT, 1], F16, tag="e2s2")
            nc.scalar.activation(e2s2[:], xls2[:], mybir.ActivationFunctionType.Exp, scale=NEG_SLOPE)
            nc.vector.tensor_tensor(out=wself2[:], in0=e1s2[:], in1=e2s2[:], op=mybir.AluOpType.max)

            # ---- layer 2
            for gi, (t0, t1) in enumerate(groups):
                SA, SB = gSA[gi], gSB[gi]
                S = SA + SB
                nt = t1 - t0
                c0 = gc0[gi]
                idxg = ixp.tile([128, S * 8], I16, tag="idx")
                nc.sync.dma_start(out=idxg[:], in_=idx_flat[:, c0 * 8:(c0 + S) * 8])
                G2 = gp.tile([128, S, T2_COLS], F16, tag="G2")
                if SA:
                    nc.gpsimd.dma_gather(
                        G2[:, 0:SA, :], T2_sh[0:b_base, :], idxg[:, 0:SA * 8],
                        128 * SA, 128 * SA, T2_COLS, queue_num=qn(), single_packet=False)
                if SB:
                    nc.gpsimd.dma_gather(
                        G2[:, SA:S, :], T2_sh[b_base:nrows, :], idxg[:, SA * 8:S * 8],
                        128 * SB, 128 * SB, T2_COLS, queue_num=qn(), single_packet=False)
                OT = otp.tile([128, S, 128], F16, tag="OT")
                nc.scalar.dma_start(out=OT[:], in_=ohT_d[:, c0:c0 + S, :])
                OH = ohp.tile([128, S, 128], F16, tag="OH")
                nc.vector.tensor_tensor(
                    out=OH[:], in0=codes[:, c0:c0 + S, None].to_broadcast([128, S, 128]),
                    in1=iota[:, None, :].to_broadcast([128, S, 128]),
                    op=mybir.AluOpType.is_equal)

                alps2 = pal.tile([128, S], F32, tag="al")
                for cs_ in range(S):
                    t = slot_tile[gi][cs_]
                    nc.tensor.matmul(out=alps2[:, cs_:cs_ + 1], lhsT=OT[:, cs_, :],
                                     rhs=alT2[:, t * 2 + 1:t * 2 + 2], start=True, stop=True)
                alDs2 = smp.tile([128, S, 1], F16, tag="alDs2")
                nc.vector.tensor_copy(out=alDs2[:], in_=alps2[:, :, None])
                xl2 = smp.tile([128, S, 1], F16, tag="xl2")
                nc.vector.tensor_tensor(out=xl2[:], in0=G2[:, :, 32:33], in1=alDs2[:],
                                        op=mybir.AluOpType.add)
                e1b = smp.tile([128, S, 1], F16, tag="e1b")
                nc.scalar.activation(e1b[:], xl2[:], mybir.ActivationFunctionType.Exp)
                e2b = smp.tile([128, S, 1], F16, tag="e2b")
                nc.scalar.activation(e2b[:], xl2[:], mybir.ActivationFunctionType.Exp, scale=NEG_SLOPE)
                rhs2 = rp.tile([128, S + nt, HID + 1], F16, tag="rhs2")
                nc.vector.tensor_tensor(out=rhs2[:, 0:S, HID:HID + 1], in0=e1b[:], in1=e2b[:],
                                        op=mybir.AluOpType.max)
                nc.vector.tensor_tensor(
                    out=rhs2[:, 0:S, 0:HID], in0=G2[:, :, 0:HID],
                    in1=rhs2[:, 0:S, HID:HID + 1].to_broadcast([128, S, HID]),
                    op=mybir.AluOpType.mult)
                nc.vector.tensor_copy(
                    out=rhs2[:, S:S + nt, HID:HID + 1], in_=wself2[:, t0:t1, :])
                nc.vector.tensor_tensor(
                    out=rhs2[:, S:S + nt, 0:HID],
                    in0=z_own[:, t0 * HID:t1 * HID].rearrange("p (t c) -> p t c", t=nt),
                    in1=wself2[:, t0:t1, :].to_broadcast([128, nt, HID]),
                    op=mybir.AluOpType.mult)

                psall2 = ep.tile([128, nt, 33], F32, tag="psall2")
                for ti, t in enumerate(range(t0, t1)):
                    ca, cb = chA[t], chB[t]
                    slots = list(range(aoff[t], aoff[t] + ca)) + \
                            list(range(SA + boff[t], SA + boff[t] + cb))
                    ps2 = ppa.tile([128, 33], F32, tag="agg")
                    for ci, cs_ in enumerate(slots):
                        nc.tensor.matmul(out=ps2[:], lhsT=OH[:, cs_, :], rhs=rhs2[:, cs_, :],
                                         start=(ci == 0), stop=False)
                    nc.tensor.matmul(out=ps2[:], lhsT=ident[:], rhs=rhs2[:, S + ti, :],
                                     start=False, stop=True)
                    nc.vector.tensor_copy(out=psall2[:, ti, :], in_=ps2[:])
                psg2_v = psall2[:]

                rec2 = smp.tile([128, nt, 1], F32, tag="rec2")
                nc.vector.reciprocal(out=rec2[:], in_=psg2_v[:, :, HID:HID + 1])
                y2 = ep.tile([128, nt, HID], F16, tag="y2")
                nc.vector.tensor_tensor(out=y2[:], in0=psg2_v[:, :, 0:HID],
                                        in1=rec2[:].to_broadcast([128, nt, HID]),
                                        op=mybir.AluOpType.mult)
                m2 = ep.tile([128, nt, HID], F16, tag="m2")
                nc.vector.tensor_scalar(out=m2[:], in0=y2[:], scalar1=0.0, scalar2=None,
                                        op0=mybir.AluOpType.min)
                nc.scalar.activation(m2[:], m2[:], mybir.ActivationFunctionType.Exp)
                nc.vector.tensor_scalar(out=y2[:], in0=y2[:], scalar1=0.0, scalar2=-1.0,
                                        op0=mybir.AluOpType.max, op1=mybir.AluOpType.add)
                h3 = m2
                nc.vector.tensor_tensor(out=h3[:], in0=m2[:], in1=y2[:], op=mybir.AluOpType.add)

                outg = ep.tile([128, nt, OUT_CH], F32, tag="outf")
                for ti, t in enumerate(range(t0, t1)):
                    pt2 = ppm.tile([128, 128], F16, tag="misc")
                    nc.tensor.transpose(out=pt2[:HID, :], in_=h3[:, ti, :], identity=ident[:])
                    h3T = ep.tile([HID, 128], F16, tag="h3T")
                    nc.vector.tensor_copy(out=h3T[:], in_=pt2[:HID, :])
                    psf = ppm.tile([128, OUT_CH], F32, tag="misc")
                    nc.tensor.matmul(out=psf[:], lhsT=h3T[:], rhs=Wout[:], start=True, stop=True)
                    nc.vector.tensor_tensor(out=outg[:, ti, :], in0=psf[:], in1=boutr[:],
                                            op=mybir.AluOpType.add)
                nc.scalar.dma_start(
                    out=out_d[t0 * 128:t1 * 128, :].rearrange("(t p) c -> p t c", p=128),
                    in_=outg[:])

    nc.compile()
    return nc


def _run(inputs, trace=False):
    meta, in_maps = _prep(**inputs)
    nc = _build(meta)
    res = run_bass_kernel_spmd(nc, in_maps, core_ids=list(range(N_CORES)), trace=trace)
    outg = np.concatenate([res.results[c]["out"] for c in range(N_CORES)], axis=0)
    out_nodes = np.empty((meta["n_pad"], OUT_CH), np.float32)
    out_nodes[meta["perm_rows"]] = outg
    return out_nodes[:meta["N"]], res


def kernel(**inputs):
    out, _ = _run(inputs, trace=False)
    return out
